# revision 1
# baseline (speedup 1.0000x reference)
"""AdaConv Trainium2 kernel — 8-core SPMD, data-parallel over batch.

Per core c (sample c):
  Stage A: kernel-prediction net for ALL 8 samples, layer-2 weights sharded
           by output channel across cores; AllToAll redistributes so each
           core ends with the full dynamic weights for its own sample.
  Stage B: build fused per-sample conv weights W_eff = PK o D and scatter
           them into block-diagonal stationary matrices S via diagonal-AP
           SBUF->SBUF DMA.
  Stage C: main grouped conv (128 groups of 4->4 ch, 3x3, reflect pad) as
           9 PSUM-accumulated float32r matmuls per 128-channel chunk.
"""
import sys
import types

sys.path.insert(0, "/opt/trn_rl_repo")

import numpy as np

import concourse.bass as bass
import concourse.mybir as mybir

N = 8          # batch == cores
CIN = 512
COUT = 512
HW = 64        # spatial
HWP = 66       # padded
NPOS = 16      # style spatial 4x4
OSL = 2048 // N      # dw2 out-channel slice per core (256)
PKSL = 2048 // N     # pk2 slice (256)
PBSL = 512 // N      # pb2 slice (64)
# AllToAll per-rank block: [dw 256*9 | pk 256 | pb 64]
BDW = 0
BPK = OSL * 9              # 2304
BPB = BPK + PKSL           # 2560
BLK = BPB + PBSL           # 2624
AG_SZ = N * BLK            # 20992

F32 = mybir.dt.float32
F32R = mybir.dt.float32r
BF16 = mybir.dt.bfloat16


# ---------------------------------------------------------------- tile patch
def _install_tile_patch():
    """walrus here rejects Drain instructions with >1 sync-wait; spread the
    Tile tail-drain waits over individual SP nops."""
    import concourse.tile as tile_mod
    from concourse.vector_clock import ScopedClock

    def _patched(self, tick_clock, wait_clock):
        nc = self.nc
        drain_inst = nc.sync.drain()
        wait_clock.add_sem_waits(
            drain_inst.ins, ScopedClock({None: tick_clock.global_clock})
        )
        waits = list(drain_inst.ins.sync_info.on_wait or [])
        if len(waits) > 1:
            drain_inst.ins.sync_info.on_wait = waits[:1]
            for w in waits[1:]:
                nop = nc.sync.nop(nofuse=True, hint="tail_wait_split")
                if nop.ins.sync_info is None:
                    nop.ins.sync_info = mybir.SyncInfo(on_wait=[w], on_update=[])
                else:
                    nop.ins.sync_info.on_wait = [w]
        nc.all_engine_barrier()
        assert self.sems is not None
        popped = nc._tile_sem_poison_stack.pop()
        assert popped is self._sem_poison
        nc.clear_and_free_semaphores(list(self.sems.allocated().values()))
        nc.all_engine_barrier()

    tile_mod.TileContext._drain_and_barrier = _patched


_install_tile_patch()
from concourse.tile import TileContext  # noqa: E402


def install_profile_shim():
    """antenv.axon_hooks is missing from this image; recreate it so
    run_bass_kernel_spmd(trace=True) can capture NTFF profiles."""
    if "antenv.axon_hooks" in sys.modules:
        return
    import antenv

    mod = types.ModuleType("antenv.axon_hooks")
    mod._hook = None
    mod.set_axon_ntff_profile_hook = lambda h: setattr(mod, "_hook", h)
    mod.get_axon_ntff_profile_hook = lambda: mod._hook
    sys.modules["antenv.axon_hooks"] = mod
    antenv.axon_hooks = mod
    try:
        if "/root/.axon_site" not in sys.path:
            sys.path.insert(0, "/root/.axon_site")
        from trn_agent_boot.trn_boot import _ntff_profile_via_ctypes

        hook = _ntff_profile_via_ctypes("/opt/axon/libaxon_pjrt.so")
        mod.set_axon_ntff_profile_hook(hook)
    except Exception:
        pass


def _ap(t_ap, offset, dims):
    """Custom flat AP over a tile's underlying tensor."""
    return bass.AP(t_ap.tensor, offset, [list(d) for d in dims])


def _pt(t):
    """Physical partition pitch (elements) of a tile."""
    return t[:, :].ap[0][0]


def _split_excess_waits(nc, max_waits=1):
    """This walrus build rejects instructions carrying more than ~1 sync-wait.
    Move excess waits onto same-engine NoOps inserted just before."""
    n_split = 0
    for f in nc.m.functions:
        for bb in f.blocks:
            newlist = []
            for inst in bb.instructions:
                si = getattr(inst, "sync_info", None)
                if si is not None and si.on_wait and len(si.on_wait) > max_waits:
                    waits = list(si.on_wait)
                    for k, w in enumerate(waits[max_waits:]):
                        nop = mybir.InstNoOp(
                            name=f"{inst.name}_ws{k}",
                            engine=inst.engine,
                            bass_nofuse=True,
                            sync_info=mybir.SyncInfo(on_wait=[w], on_update=[]),
                        )
                        newlist.append(nop)
                        n_split += 1
                    si.on_wait = waits[:max_waits]
                newlist.append(inst)
            try:
                bb.instructions[:] = newlist
            except TypeError:
                bb.set_instructions(newlist)
    return n_split


def build_nc():
    nc = bass.Bass(target_bir_lowering=False)

    style = nc.declare_dram_parameter("style_all", [CIN, N * NPOS], F32, isOutput=False)
    xin = nc.declare_dram_parameter("xin", [CIN, HW * HW], F32, isOutput=False)
    w1t = nc.declare_dram_parameter("w1t", [CIN, CIN], F32, isOutput=False)
    pk1t = nc.declare_dram_parameter("pk1t", [CIN, CIN], F32, isOutput=False)
    pb1t = nc.declare_dram_parameter("pb1t", [CIN, CIN], F32, isOutput=False)
    w2t = nc.declare_dram_parameter("w2t", [16, 128, OSL], F32, isOutput=False)
    pk2t = nc.declare_dram_parameter("pk2t", [4, 128, PKSL], F32, isOutput=False)
    pb2t = nc.declare_dram_parameter("pb2t", [4, 128, PBSL], F32, isOutput=False)
    b1 = nc.declare_dram_parameter("b1", [CIN], F32, isOutput=False)
    bk1 = nc.declare_dram_parameter("bk1", [CIN], F32, isOutput=False)
    bb1 = nc.declare_dram_parameter("bb1", [CIN], F32, isOutput=False)
    b2s = nc.declare_dram_parameter("b2s", [OSL], F32, isOutput=False)
    bk2s = nc.declare_dram_parameter("bk2s", [PKSL], F32, isOutput=False)
    bb2s = nc.declare_dram_parameter("bb2s", [PBSL], F32, isOutput=False)
    perm = nc.declare_dram_parameter("perm", [4, 128, 128], F32, isOutput=False)
    ident = nc.declare_dram_parameter("ident", [128, 128], F32, isOutput=False)
    selm = nc.declare_dram_parameter("selm", [36, 9 * 128], F32, isOutput=False)
    maskm = nc.declare_dram_parameter("maskm", [128, 128], F32, isOutput=False)
    out = nc.declare_dram_parameter("out", [COUT, HW * HW], F32, isOutput=True)

    with TileContext(nc) as tc:
        with (
            tc.tile_pool(name="sb", bufs=1) as sb,
            tc.tile_pool(name="sbx", bufs=1) as sbx,
            tc.tile_pool(name="sbo", bufs=2) as sbo,
            tc.tile_pool(name="ps", bufs=2, space="PSUM") as ps,
            tc.tile_pool(name="psc", bufs=4, space="PSUM") as psc,
            tc.tile_pool(name="dram", bufs=1, space="DRAM") as dram,
        ):
            # ---------------- stage C input loads first (prefetch)
            xp = [sbx.tile([128, 4384], F32, tag=f"xp{ch}", name=f"xp{ch}") for ch in range(4)]
            for ch in range(4):
                dst = _ap(xp[ch], HWP + 1, [[4384, 128], [HWP, HW], [1, HW]])
                nc.sync.dma_start(out=dst, in_=xin[ch * 128:(ch + 1) * 128, :])

            # ---------------- stage A: layer-1 (h = lrelu(W1 s + b1))
            st = []
            for i in range(4):
                t = sb.tile([128, N * NPOS], F32, tag=f"st{i}", name=f"st{i}")
                nc.sync.dma_start(out=t[:, :], in_=style[i * 128:(i + 1) * 128, :])
                st.append(t)
            w1sb = sb.tile([128, 4 * CIN], F32, tag="w1sb", name="w1sb")
            for it in range(4):
                nc.sync.dma_start(
                    out=_ap(w1sb, it * CIN, [[4 * CIN, 128], [1, CIN]]),
                    in_=w1t[it * 128:(it + 1) * 128, :],
                )
            b1t = sb.tile([128, 4], F32, tag="b1t", name="b1t")
            for ot in range(4):
                nc.sync.dma_start(
                    out=b1t[:, ot:ot + 1], in_=_ap(b1[:], ot * 128, [[1, 128], [1, 1]])
                )
            h = [sb.tile([128, N * NPOS], F32, tag=f"h{ot}", name=f"h{ot}") for ot in range(4)]
            for ot in range(4):
                hp = ps.tile([128, N * NPOS], F32, tag="sA", name="sA")
                for it in range(4):
                    nc.tensor.matmul(
                        hp[:, :],
                        _ap(w1sb, it * CIN + ot * 128, [[4 * CIN, 128], [1, 128]]),
                        st[it][:, :],
                        start=(it == 0),
                        stop=(it == 3),
                    )
                nc.scalar.activation(
                    h[ot][:, :], hp[:, :], mybir.ActivationFunctionType.Identity,
                    bias=b1t[:, ot:ot + 1],
                )
                zt = sb.tile([128, N * NPOS], F32, tag="zt", name="zt")
                nc.vector.tensor_scalar_mul(zt[:, :], h[ot][:, :], 0.01)
                nc.vector.tensor_max(h[ot][:, :], h[ot][:, :], zt[:, :])

            # ---------------- stage A: dw2 slice for all samples
            w2sb = sb.tile([128, 16 * OSL], F32, tag="w2sb", name="w2sb")
            nc.sync.dma_start(
                out=_ap(w2sb, 0, [[16 * OSL, 128], [OSL, 16], [1, OSL]]),
                in_=_ap(w2t[:, :, :], 0, [[OSL, 128], [128 * OSL, 16], [1, OSL]]),
            )
            b2t = sb.tile([128, 2], F32, tag="b2t", name="b2t")
            for o2 in range(2):
                nc.sync.dma_start(
                    out=b2t[:, o2:o2 + 1],
                    in_=_ap(b2s[:], o2 * 128, [[1, 128], [1, 1]]),
                )
            agin = dram.tile([AG_SZ], F32)
            agout = dram.tile([AG_SZ], F32)
            dwc = [sb.tile([128, 96], F32, tag=f"dwc{o2}", name=f"dwc{o2}") for o2 in range(2)]
            for o2 in range(2):
                dps = ps.tile([128, N * 9], F32, tag="sA", name="sA")
                for kt in range(16):
                    it, tap = kt // 4, kt % 4
                    di, dj = tap // 2, tap % 2
                    rhs = _ap(h[it], di * 4 + dj,
                              [[N * NPOS, 128], [NPOS, N], [4, 3], [1, 3]])
                    nc.tensor.matmul(
                        dps[:, :],
                        _ap(w2sb, kt * OSL + o2 * 128, [[16 * OSL, 128], [1, 128]]),
                        rhs,
                        start=(kt == 0),
                        stop=(kt == 15),
                    )
                nc.scalar.activation(
                    dwc[o2][:, 0:72], dps[:, :], mybir.ActivationFunctionType.Identity,
                    bias=b2t[:, o2:o2 + 1],
                )
                # agin[n*BLK + (o2*128+p)*9 + pos] = dwc[o2][p, n*9+pos]
                nc.sync.dma_start(
                    out=_ap(agin[:], o2 * 128 * 9, [[9, 128], [BLK, N], [1, 9]]),
                    in_=_ap(dwc[o2], 0, [[96, 128], [9, N], [1, 9]]),
                )

            # ---------------- stage A: pooled-style path (pk / pb)
            sp = [sb.tile([128, N], F32, tag=f"sp{i}", name=f"sp{i}") for i in range(4)]
            for i in range(4):
                nc.vector.tensor_reduce(
                    sp[i][:, :],
                    _ap(st[i], 0, [[N * NPOS, 128], [NPOS, N], [1, NPOS]]),
                    axis=mybir.AxisListType.X,
                    op=mybir.AluOpType.add,
                )
                nc.vector.tensor_scalar_mul(sp[i][:, :], sp[i][:, :], 1.0 / NPOS)

            def layer1(wt_param, bias_param, tagp):
                wsb = sb.tile([128, 4 * CIN], F32, tag=f"{tagp}w", name=f"{tagp}w")
                for it in range(4):
                    nc.sync.dma_start(
                        out=_ap(wsb, it * CIN, [[4 * CIN, 128], [1, CIN]]),
                        in_=wt_param[it * 128:(it + 1) * 128, :],
                    )
                bt = sb.tile([128, 4], F32, tag=f"{tagp}b", name=f"{tagp}b")
                for ot in range(4):
                    nc.sync.dma_start(
                        out=bt[:, ot:ot + 1],
                        in_=_ap(bias_param[:], ot * 128, [[1, 128], [1, 1]]),
                    )
                acts = []
                for ot in range(4):
                    ap_ = ps.tile([128, N], F32, tag="sA", name="sA")
                    for it in range(4):
                        nc.tensor.matmul(
                            ap_[:, :],
                            _ap(wsb, it * CIN + ot * 128, [[4 * CIN, 128], [1, 128]]),
                            sp[it][:, :],
                            start=(it == 0),
                            stop=(it == 3),
                        )
                    a = sb.tile([128, N], F32, tag=f"{tagp}a{ot}", name=f"{tagp}a{ot}")
                    nc.scalar.activation(
                        a[:, :], ap_[:, :], mybir.ActivationFunctionType.Identity,
                        bias=bt[:, ot:ot + 1],
                    )
                    zt2 = sb.tile([128, N], F32, tag="zt2", name="zt2")
                    nc.vector.tensor_scalar_mul(zt2[:, :], a[:, :], 0.01)
                    nc.vector.tensor_max(a[:, :], a[:, :], zt2[:, :])
                    acts.append(a)
                return acts

            a1 = layer1(pk1t, bk1, "pk1")
            c1 = layer1(pb1t, bb1, "pb1")

            pk2sb = sb.tile([128, 4 * PKSL], F32, tag="pk2sb", name="pk2sb")
            nc.sync.dma_start(
                out=_ap(pk2sb, 0, [[4 * PKSL, 128], [PKSL, 4], [1, PKSL]]),
                in_=_ap(pk2t[:, :, :], 0, [[PKSL, 128], [128 * PKSL, 4], [1, PKSL]]),
            )
            bk2t = sb.tile([128, 2], F32, tag="bk2t", name="bk2t")
            for o2 in range(2):
                nc.sync.dma_start(
                    out=bk2t[:, o2:o2 + 1],
                    in_=_ap(bk2s[:], o2 * 128, [[1, 128], [1, 1]]),
                )
            for o2 in range(2):
                pp = ps.tile([128, N], F32, tag="sA", name="sA")
                for it in range(4):
                    nc.tensor.matmul(
                        pp[:, :],
                        _ap(pk2sb, it * PKSL + o2 * 128, [[4 * PKSL, 128], [1, 128]]),
                        a1[it][:, :],
                        start=(it == 0),
                        stop=(it == 3),
                    )
                pkc = sb.tile([128, 32], F32, tag=f"pkc{o2}", name=f"pkc{o2}")
                nc.scalar.activation(
                    pkc[:, 0:8], pp[:, :], mybir.ActivationFunctionType.Identity,
                    bias=bk2t[:, o2:o2 + 1],
                )
                nc.sync.dma_start(
                    out=_ap(agin[:], BPK + o2 * 128, [[1, 128], [BLK, N]]),
                    in_=_ap(pkc, 0, [[32, 128], [1, N]]),
                )

            pb2sb = sb.tile([128, 4 * PBSL], F32, tag="pb2sb", name="pb2sb")
            nc.sync.dma_start(
                out=_ap(pb2sb, 0, [[4 * PBSL, 128], [PBSL, 4], [1, PBSL]]),
                in_=_ap(pb2t[:, :, :], 0, [[PBSL, 128], [128 * PBSL, 4], [1, PBSL]]),
            )
            bb2t = sb.tile([64, 1], F32, tag="bb2t", name="bb2t")
            nc.sync.dma_start(
                out=bb2t[:, 0:1], in_=_ap(bb2s[:], 0, [[1, 64], [1, 1]])
            )
            pbp = ps.tile([64, N], F32, tag="sA", name="sA")
            for it in range(4):
                nc.tensor.matmul(
                    pbp[:, :],
                    _ap(pb2sb, it * PBSL, [[4 * PBSL, 128], [1, PBSL]]),
                    c1[it][:, :],
                    start=(it == 0),
                    stop=(it == 3),
                )
            pbc = sb.tile([64, 32], F32, tag="pbc", name="pbc")
            nc.scalar.activation(
                pbc[:, 0:8], pbp[:, :], mybir.ActivationFunctionType.Identity,
                bias=bb2t[:, 0:1],
            )
            nc.sync.dma_start(
                out=_ap(agin[:], BPB, [[1, 64], [BLK, N]]),
                in_=_ap(pbc, 0, [[32, 64], [1, N]]),
            )

            # ---------------- AllToAll: core c receives, from every rank r,
            # rank r's o-slice of sample c's dynamic weights.
            nc.gpsimd.collective_compute(
                "AllToAll",
                mybir.AluOpType.bypass,
                replica_groups=[list(range(N))],
                ins=[agin[:].opt()],
                outs=[agout[:].opt()],
            )

            # ---------------- stage B: own-sample weight assembly
            D = [sb.tile([128, 64], F32, tag=f"D{ch}", name=f"D{ch}") for ch in range(4)]
            PK = [sb.tile([128, 32], F32, tag=f"PK{ch}", name=f"PK{ch}") for ch in range(4)]
            PB = [sb.tile([128, 32], F32, tag=f"PB{ch}", name=f"PB{ch}") for ch in range(4)]
            for ch in range(4):
                for half in range(2):
                    r = 2 * ch + half
                    base = r * BLK
                    nc.sync.dma_start(
                        out=_ap(D[ch], half * 64 * 64, [[64, 64], [9, 4], [1, 9]]),
                        in_=_ap(agout[:], base, [[36, 64], [9, 4], [1, 9]]),
                    )
                    nc.sync.dma_start(
                        out=_ap(PK[ch], half * 64 * 32, [[32, 64], [1, 4]]),
                        in_=_ap(agout[:], base + BPK, [[4, 64], [1, 4]]),
                    )
                    nc.sync.dma_start(
                        out=_ap(PB[ch], half * 64 * 32, [[32, 64], [1, 1]]),
                        in_=_ap(agout[:], base + BPB, [[1, 64], [1, 1]]),
                    )

            permsb = sb.tile([128, 512], F32, tag="permsb", name="permsb")
            nc.sync.dma_start(
                out=_ap(permsb, 0, [[512, 128], [128, 4], [1, 128]]),
                in_=_ap(perm[:, :, :], 0, [[128, 128], [128 * 128, 4], [1, 128]]),
            )
            identsb = sb.tile([128, 128], F32, tag="identsb", name="identsb")
            nc.sync.dma_start(out=identsb[:, :], in_=ident[:, :])
            selsb = sb.tile([36, 9 * 128], F32, tag="selsb", name="selsb")
            nc.sync.dma_start(out=selsb[:, :], in_=selm[:, :])
            masksb = sb.tile([128, 128], F32, tag="masksb", name="masksb")
            nc.sync.dma_start(out=masksb[:, :], in_=maskm[:, :])
            S = [sb.tile([128, 9 * 128], BF16, tag=f"S{ch}", name=f"S{ch}") for ch in range(4)]
            wef = [sb.tile([128, 64], F32, tag=f"wef{ch}", name=f"wef{ch}") for ch in range(4)]
            wefT = [sb.tile([36, 128], F32, tag=f"wefT{ch}", name=f"wefT{ch}") for ch in range(4)]
            for ch in range(4):
                dp = ps.tile([128, 144], F32, tag="sA", name="sA")
                for m2 in range(4):
                    nc.tensor.matmul(
                        dp[:, m2 * 36:(m2 + 1) * 36],
                        permsb[:, m2 * 128:(m2 + 1) * 128],
                        D[ch][:, 0:36],
                        start=True,
                        stop=True,
                    )
                tmp = sb.tile([128, 36], F32, tag="weftmp", name="weftmp")
                nc.vector.tensor_scalar_mul(
                    wef[ch][:, 0:36], dp[:, 0:36], PK[ch][:, 0:1]
                )
                for m2 in range(1, 4):
                    nc.vector.tensor_scalar_mul(
                        tmp[:, :], dp[:, m2 * 36:(m2 + 1) * 36], PK[ch][:, m2:m2 + 1]
                    )
                    nc.vector.tensor_add(wef[ch][:, 0:36], wef[ch][:, 0:36], tmp[:, :])
                # expand W_eff -> block-diag S via PE select-matmuls + mask
                tp = ps.tile([36, 128], F32, tag="sA", name="sA")
                nc.tensor.matmul(
                    tp[:, :], wef[ch][:, 0:36], identsb[:, :], is_transpose=True,
                    start=True, stop=True,
                )
                nc.vector.tensor_copy(wefT[ch][:, :], tp[:, :])
                for t in range(9):
                    sps = ps.tile([128, 128], F32, tag="sB", name="sB")
                    nc.tensor.matmul(
                        sps[:, :],
                        selsb[:, t * 128:(t + 1) * 128],
                        wefT[ch][:, :],
                        start=True, stop=True,
                    )
                    nc.vector.tensor_tensor(
                        S[ch][:, t * 128:(t + 1) * 128], sps[:, :], masksb[:, :],
                        op=mybir.AluOpType.mult,
                    )

            # ---------------- stage C: reflect-pad edges + conv
            for ch in range(4):
                nc.vector.tensor_copy(
                    _ap(xp[ch], 1, [[4384, 128], [1, HW]]),
                    _ap(xp[ch], 2 * HWP + 1, [[4384, 128], [1, HW]]),
                )
                nc.vector.tensor_copy(
                    _ap(xp[ch], 65 * HWP + 1, [[4384, 128], [1, HW]]),
                    _ap(xp[ch], 63 * HWP + 1, [[4384, 128], [1, HW]]),
                )
                nc.vector.tensor_copy(
                    _ap(xp[ch], 0, [[4384, 128], [HWP, HWP]]),
                    _ap(xp[ch], 2, [[4384, 128], [HWP, HWP]]),
                )
                nc.vector.tensor_copy(
                    _ap(xp[ch], 65, [[4384, 128], [HWP, HWP]]),
                    _ap(xp[ch], 63, [[4384, 128], [HWP, HWP]]),
                )

            xb = [sbx.tile([128, 4384], BF16, tag=f"xb{ch}", name=f"xb{ch}")
                  for ch in range(4)]
            for ch in range(4):
                eng = nc.vector if ch % 2 == 0 else nc.scalar
                if ch % 2 == 0:
                    nc.vector.tensor_copy(xb[ch][:, :], xp[ch][:, :])
                else:
                    nc.scalar.activation(
                        xb[ch][:, :], xp[ch][:, :],
                        mybir.ActivationFunctionType.Copy,
                    )
            for ch in range(4):
                osb = sbo.tile([128, HW * HW], F32, tag="osb", name="osb")
                for sub in range(8):
                    cps = psc.tile([128, 512], F32, tag="cps", name="cps")
                    r0 = sub * 8
                    for tap in range(9):
                        di, dj = tap // 3, tap % 3
                        lhs = S[ch][:, tap * 128:(tap + 1) * 128]
                        rhs = _ap(xb[ch], (r0 + di) * HWP + dj,
                                  [[4384, 128], [HWP, 8], [1, HW]])
                        nc.tensor.matmul(
                            cps[:, :],
                            lhs,
                            rhs,
                            start=(tap == 0),
                            stop=(tap == 8),
                        )
                    nc.scalar.activation(
                        osb[:, r0 * HW:(r0 + 8) * HW], cps[:, :],
                        mybir.ActivationFunctionType.Identity,
                        bias=PB[ch][:, 0:1],
                    )
                nc.sync.dma_start(
                    out=out[ch * 128:(ch + 1) * 128, :], in_=osb[:, :]
                )

    _split_excess_waits(nc)
    return nc


_NC_CACHE = {}


def _get_nc():
    if "nc" not in _NC_CACHE:
        _NC_CACHE["nc"] = build_nc()
    return _NC_CACHE["nc"]


def make_in_maps(inputs):
    """Host-side shard/layout prep (pure layout: transpose/reshape/slice)."""
    style = np.asarray(inputs["style_encoding"], np.float32)
    pred = np.asarray(inputs["predicted"], np.float32)
    w1 = np.asarray(inputs["dw1_w"], np.float32).reshape(512, 512)
    w2 = np.asarray(inputs["dw2_w"], np.float32).reshape(2048, 512, 2, 2)
    pk1 = np.asarray(inputs["pk1_w"], np.float32).reshape(512, 512)
    pk2 = np.asarray(inputs["pk2_w"], np.float32).reshape(2048, 512)
    pb1 = np.asarray(inputs["pb1_w"], np.float32).reshape(512, 512)
    pb2 = np.asarray(inputs["pb2_w"], np.float32).reshape(512, 512)

    w1t = np.ascontiguousarray(w1.T)
    pk1t = np.ascontiguousarray(pk1.T)
    pb1t = np.ascontiguousarray(pb1.T)
    w2t_full = (
        w2.reshape(2048, 4, 128, 2, 2)
        .transpose(1, 3, 4, 2, 0)          # [it, di, dj, 128, o]
        .reshape(16, 128, 2048)
    )
    pk2t_full = np.ascontiguousarray(pk2.T).reshape(4, 128, 2048)
    pb2t_full = np.ascontiguousarray(pb2.T).reshape(4, 128, 512)
    st_all = np.ascontiguousarray(
        style.transpose(1, 0, 2, 3).reshape(512, N * NPOS)
    )

    permm = np.zeros((4, 128, 128), np.float32)
    for m2 in range(4):
        for p in range(128):
            permm[m2, 4 * (p // 4) + m2, p] = 1.0
    identm = np.eye(128, dtype=np.float32)
    selm = np.zeros((36, 9, 128), np.float32)
    for t in range(9):
        for p in range(128):
            selm[(p % 4) * 9 + t, t, p] = 1.0
    selm = selm.reshape(36, 9 * 128)
    maskm = np.zeros((128, 128), np.float32)
    for p in range(128):
        for col in range(128):
            if p // 4 == col // 4:
                maskm[p, col] = 1.0

    in_maps = []
    for c in range(N):
        m = {
            "style_all": st_all,
            "xin": np.ascontiguousarray(pred[c].reshape(512, HW * HW)),
            "w1t": w1t,
            "pk1t": pk1t,
            "pb1t": pb1t,
            "w2t": np.ascontiguousarray(w2t_full[:, :, c * OSL:(c + 1) * OSL]),
            "pk2t": np.ascontiguousarray(pk2t_full[:, :, c * PKSL:(c + 1) * PKSL]),
            "pb2t": np.ascontiguousarray(pb2t_full[:, :, c * PBSL:(c + 1) * PBSL]),
            "b1": np.asarray(inputs["dw1_b"], np.float32),
            "bk1": np.asarray(inputs["pk1_b"], np.float32),
            "bb1": np.asarray(inputs["pb1_b"], np.float32),
            "b2s": np.asarray(inputs["dw2_b"], np.float32)[c * OSL:(c + 1) * OSL],
            "bk2s": np.asarray(inputs["pk2_b"], np.float32)[c * PKSL:(c + 1) * PKSL],
            "bb2s": np.asarray(inputs["pb2_b"], np.float32)[c * PBSL:(c + 1) * PBSL],
            "perm": permm,
            "ident": identm,
            "selm": selm,
            "maskm": maskm,
        }
        in_maps.append(m)
    return in_maps


def kernel(**inputs):
    install_profile_shim()
    from concourse.bass_utils import run_bass_kernel_spmd

    nc = _get_nc()
    in_maps = make_in_maps(inputs)
    res = run_bass_kernel_spmd(nc, in_maps, core_ids=list(range(N)))
    outs = [np.asarray(res.results[c]["out"]).reshape(COUT, HW, HW)
            for c in range(N)]
    return np.stack(outs, axis=0).astype(np.float32)



# revision 5
# speedup vs baseline: 1.6774x; 1.6774x over previous
"""AdaConv Trainium2 kernel — 8-core SPMD, data-parallel over batch.

v2 redesign vs baseline:
  * All dynamic tensors fp16 on device (matmuls run at full PE rate; the
    baseline ran stage A in f32 at 1/4 rate). Output fp16, upcast on host.
  * Input image is reflect-padded + chunked on the HOST, so the device
    load is 128 fat contiguous DMA descriptors per chunk instead of the
    baseline's ~32k row-sized descriptors.
  * Stage A (kernel-prediction net, layer-2 sharded by out-channel):
    leaky-relu fused into the psum-drain via the Lrelu activation; dw2
    computed weights-as-moving so the AllToAll payload is written with
    72 fat descriptors; pk2/pb2 merged into one [8, 320] psum tile
    written with 8 descriptors. Static biases of the sharded layers are
    folded in on the RECEIVER side from host constants (they don't need
    the collective).
  * Stage B assembles block-diag stationaries S[ch] (fp16) via
    perm-matmuls + per-partition scales + select-matmuls + mask.
  * Stage C: grouped 3x3 conv as 9 psum-accumulated fp16 matmuls per
    128-channel chunk, tap-outer so the PE weight reorder window hides
    LDWEIGHTS; 2-sub waves over 4 rotating psum banks.
"""
import sys
import types

sys.path.insert(0, "/opt/trn_rl_repo")

import numpy as np

import concourse.bass as bass
import concourse.mybir as mybir

N = 8          # batch == cores
CIN = 512
COUT = 512
HW = 64        # spatial
HWP = 66       # padded
XPW = 4384     # padded free width (66*66=4356 used)
NPOS = 16      # style spatial 4x4
OSL = 2048 // N      # dw2 out-channel slice per core (256)
PKSL = 2048 // N     # pk2 slice (256)
PBSL = 512 // N      # pb2 slice (64)
# AllToAll per-rank block (fp16 elems): [dw 9*256 | pk 256 | pb 64]
BPK = 9 * OSL              # 2304
BPB = BPK + PKSL           # 2560
BLK = BPB + PBSL           # 2624
AG_SZ = N * BLK            # 20992

F32 = mybir.dt.float32
F16 = mybir.dt.float16


# ---------------------------------------------------------------- tile patch
def _install_tile_patch():
    """walrus here rejects Drain instructions with >1 sync-wait; spread the
    Tile tail-drain waits over individual SP nops."""
    import concourse.tile as tile_mod
    from concourse.vector_clock import ScopedClock

    def _patched(self, tick_clock, wait_clock):
        nc = self.nc
        drain_inst = nc.sync.drain()
        wait_clock.add_sem_waits(
            drain_inst.ins, ScopedClock({None: tick_clock.global_clock})
        )
        waits = list(drain_inst.ins.sync_info.on_wait or [])
        if len(waits) > 1:
            drain_inst.ins.sync_info.on_wait = waits[:1]
            for w in waits[1:]:
                nop = nc.sync.nop(nofuse=True, hint="tail_wait_split")
                if nop.ins.sync_info is None:
                    nop.ins.sync_info = mybir.SyncInfo(on_wait=[w], on_update=[])
                else:
                    nop.ins.sync_info.on_wait = [w]
        nc.all_engine_barrier()
        assert self.sems is not None
        popped = nc._tile_sem_poison_stack.pop()
        assert popped is self._sem_poison
        nc.clear_and_free_semaphores(list(self.sems.allocated().values()))
        nc.all_engine_barrier()

    tile_mod.TileContext._drain_and_barrier = _patched


_install_tile_patch()
from concourse.tile import TileContext  # noqa: E402


def install_profile_shim():
    """antenv.axon_hooks is missing from this image; recreate it so
    run_bass_kernel_spmd(trace=True) can capture NTFF profiles."""
    if "antenv.axon_hooks" in sys.modules:
        return
    import antenv

    mod = types.ModuleType("antenv.axon_hooks")
    mod._hook = None
    mod.set_axon_ntff_profile_hook = lambda h: setattr(mod, "_hook", h)
    mod.get_axon_ntff_profile_hook = lambda: mod._hook
    sys.modules["antenv.axon_hooks"] = mod
    antenv.axon_hooks = mod
    try:
        if "/root/.axon_site" not in sys.path:
            sys.path.insert(0, "/root/.axon_site")
        from trn_agent_boot.trn_boot import _ntff_profile_via_ctypes

        hook = _ntff_profile_via_ctypes("/opt/axon/libaxon_pjrt.so")
        mod.set_axon_ntff_profile_hook(hook)
    except Exception:
        pass


def _ap(t_ap, offset, dims):
    """Custom flat AP over a tile's underlying tensor."""
    return bass.AP(t_ap.tensor, offset, [list(d) for d in dims])


def _pt(t):
    """Physical partition pitch (elements) of a tile."""
    return t[:, :].ap[0][0]


def _split_excess_waits(nc, max_waits=1):
    """This walrus build rejects instructions carrying more than ~1 sync-wait.
    Move excess waits onto same-engine NoOps inserted just before."""
    n_split = 0
    for f in nc.m.functions:
        for bb in f.blocks:
            newlist = []
            for inst in bb.instructions:
                si = getattr(inst, "sync_info", None)
                if si is not None and si.on_wait and len(si.on_wait) > max_waits:
                    waits = list(si.on_wait)
                    for k, w in enumerate(waits[max_waits:]):
                        nop = mybir.InstNoOp(
                            name=f"{inst.name}_ws{k}",
                            engine=inst.engine,
                            bass_nofuse=True,
                            sync_info=mybir.SyncInfo(on_wait=[w], on_update=[]),
                        )
                        newlist.append(nop)
                        n_split += 1
                    si.on_wait = waits[:max_waits]
                newlist.append(inst)
            try:
                bb.instructions[:] = newlist
            except TypeError:
                bb.set_instructions(newlist)
    return n_split


LRELU = mybir.ActivationFunctionType.Lrelu
IDENT = mybir.ActivationFunctionType.Identity
COPY = mybir.ActivationFunctionType.Copy


def build_nc():
    nc = bass.Bass(target_bir_lowering=False)

    xpad = nc.declare_dram_parameter("xpad", [CIN, XPW], F16, isOutput=False)
    style = nc.declare_dram_parameter("style", [CIN, N * NPOS], F16, isOutput=False)
    w1t = nc.declare_dram_parameter("w1t", [CIN, CIN], F16, isOutput=False)
    pw1t = nc.declare_dram_parameter("pw1t", [CIN, 2 * CIN], F16, isOutput=False)
    w2m = nc.declare_dram_parameter("w2m", [128, 16 * OSL], F16, isOutput=False)
    pwm = nc.declare_dram_parameter("pwm", [128, 4 * (PKSL + PBSL)], F16, isOutput=False)
    b1t = nc.declare_dram_parameter("b1t", [128, 4], F16, isOutput=False)
    pwb1 = nc.declare_dram_parameter("pwb1", [128, 8], F16, isOutput=False)
    biasD = nc.declare_dram_parameter("biasD", [128, 4 * 36], F16, isOutput=False)
    biasPK = nc.declare_dram_parameter("biasPK", [128, 4 * 4], F16, isOutput=False)
    biasPB = nc.declare_dram_parameter("biasPB", [128, 4], F16, isOutput=False)
    selm = nc.declare_dram_parameter("selm", [36, 9 * 128], F16, isOutput=False)
    maskm = nc.declare_dram_parameter("maskm", [128, 3 * 128], F16, isOutput=False)
    permm = nc.declare_dram_parameter("permm", [128, 4 * 128], F16, isOutput=False)
    ident = nc.declare_dram_parameter("ident", [128, 128], F16, isOutput=False)
    out = nc.declare_dram_parameter("out", [COUT, HW * HW], F16, isOutput=True)

    with TileContext(nc) as tc:
        with (
            tc.tile_pool(name="sb", bufs=1) as sb,
            tc.tile_pool(name="sbx", bufs=1) as sbx,
            tc.tile_pool(name="sbo", bufs=2) as sbo,
            tc.tile_pool(name="psb", bufs=2, space="PSUM") as psb,
            tc.tile_pool(name="psc", bufs=4, space="PSUM") as psc,
            tc.tile_pool(name="dram", bufs=1, space="DRAM") as dram,
        ):
            # ------------ stage-A-critical loads first
            st = []
            for i in range(4):
                t = sb.tile([128, N * NPOS], F16, tag=f"st{i}", name=f"st{i}")
                nc.sync.dma_start(out=t[:, :], in_=style[i * 128:(i + 1) * 128, :])
                st.append(t)
            w1sb = sb.tile([128, 4 * CIN], F16, tag="w1sb", name="w1sb")
            for it in range(4):
                nc.sync.dma_start(
                    out=_ap(w1sb, it * CIN, [[4 * CIN, 128], [1, CIN]]),
                    in_=w1t[it * 128:(it + 1) * 128, :],
                )
            b1tt = sb.tile([128, 4], F16, tag="b1tt", name="b1tt")
            nc.sync.dma_start(out=b1tt[:, :], in_=b1t[:, :])
            w2msb = sb.tile([128, 16 * OSL], F16, tag="w2msb", name="w2msb")
            nc.sync.dma_start(out=w2msb[:, :], in_=w2m[:, :])
            pw1sb = sb.tile([128, 4 * 2 * CIN], F16, tag="pw1sb", name="pw1sb")
            for it in range(4):
                nc.sync.dma_start(
                    out=_ap(pw1sb, it * 2 * CIN, [[4 * 2 * CIN, 128], [1, 2 * CIN]]),
                    in_=pw1t[it * 128:(it + 1) * 128, :],
                )
            pwb1t = sb.tile([128, 8], F16, tag="pwb1t", name="pwb1t")
            nc.sync.dma_start(out=pwb1t[:, :], in_=pwb1[:, :])
            pwmsb = sb.tile([128, 4 * 320], F16, tag="pwmsb", name="pwmsb")
            nc.sync.dma_start(out=pwmsb[:, :], in_=pwm[:, :])

            # ------------ stage C input prefetch (fat descriptors)
            xp = [sbx.tile([128, XPW], F16, tag=f"xp{ch}", name=f"xp{ch}")
                  for ch in range(4)]
            for ch in range(4):
                nc.sync.dma_start(
                    out=xp[ch][:, :], in_=xpad[ch * 128:(ch + 1) * 128, :]
                )

            # ------------ stage B constants
            permsb = sb.tile([128, 4 * 128], F16, tag="permsb", name="permsb")
            nc.sync.dma_start(out=permsb[:, :], in_=permm[:, :])
            identsb = sb.tile([128, 128], F16, tag="identsb", name="identsb")
            nc.sync.dma_start(out=identsb[:, :], in_=ident[:, :])
            selsb = sb.tile([36, 9 * 128], F16, tag="selsb", name="selsb")
            nc.sync.dma_start(out=selsb[:, :], in_=selm[:, :])
            masksb = sb.tile([128, 3 * 128], F16, tag="masksb", name="masksb")
            nc.sync.dma_start(out=masksb[:, :], in_=maskm[:, :])
            biasDsb = sb.tile([128, 4 * 36], F16, tag="biasDsb", name="biasDsb")
            nc.sync.dma_start(out=biasDsb[:, :], in_=biasD[:, :])
            biasPKsb = sb.tile([128, 16], F16, tag="biasPKsb", name="biasPKsb")
            nc.sync.dma_start(out=biasPKsb[:, :], in_=biasPK[:, :])
            biasPBsb = sb.tile([128, 4], F16, tag="biasPBsb", name="biasPBsb")
            nc.sync.dma_start(out=biasPBsb[:, :], in_=biasPB[:, :])

            # ------------ stage A: h = lrelu(W1 s + b1), drained directly
            # into im2col layout h2[ot][:, dydx*72:+72] = (n, ty, tx) windows
            h2 = [sb.tile([128, 4 * 72], F16, tag=f"h2{ot}", name=f"h2{ot}")
                  for ot in range(4)]
            for ot in range(4):
                pa = psb.tile([128, 128], F32, tag="sA", name="pa")
                for it in range(4):
                    nc.tensor.matmul(
                        pa[:, :],
                        _ap(w1sb, it * CIN + ot * 128, [[4 * CIN, 128], [1, 128]]),
                        st[it][:, :],
                        start=(it == 0),
                        stop=(it == 3),
                    )
                pap = _pt(pa)
                for dy in range(2):
                    for dx in range(2):
                        nc.scalar.activation(
                            h2[ot][:, (dy * 2 + dx) * 72:(dy * 2 + dx + 1) * 72],
                            _ap(pa, dy * 4 + dx,
                                [[pap, 128], [NPOS, N], [4, 3], [1, 3]]),
                            LRELU,
                            bias=b1tt[:, ot:ot + 1], alpha=0.01,
                        )

            # ------------ stage A: dw2 slice, weights-as-moving
            # psum [72=(n,ty,tx), 256=o-slice]
            agin = dram.tile([AG_SZ], F16)
            agout = dram.tile([AG_SZ], F16)
            pd = psb.tile([72, OSL], F32, tag="sA", name="pd")
            k = 0
            for ib in range(4):
                for dydx in range(4):
                    nc.tensor.matmul(
                        pd[:, :],
                        h2[ib][:, dydx * 72:(dydx + 1) * 72],
                        w2msb[:, (ib * 4 + dydx) * OSL:
                              (ib * 4 + dydx + 1) * OSL],
                        start=(k == 0),
                        stop=(k == 15),
                    )
                    k += 1
            dwt = sb.tile([72, OSL], F16, tag="dwt", name="dwt")
            nc.scalar.activation(dwt[:, :], pd[:, :], COPY)
            for n in range(N):
                nc.sync.dma_start(
                    out=_ap(agin[:], n * BLK, [[OSL, 9], [1, OSL]]),
                    in_=dwt[9 * n:9 * n + 9, :],
                )

            # ------------ stage A: pooled path (pk1|pb1 then pk2|pb2)
            sp = [sb.tile([128, N], F16, tag=f"sp{i}", name=f"sp{i}")
                  for i in range(4)]
            with nc.allow_low_precision("16-term style pool in fp16"):
                for i in range(4):
                    nc.vector.tensor_reduce(
                        sp[i][:, :],
                        _ap(st[i], 0, [[_pt(st[i]), 128], [NPOS, N], [1, NPOS]]),
                        axis=mybir.AxisListType.X,
                        op=mybir.AluOpType.add,
                    )
            ac = []
            for po in range(8):
                pp = psb.tile([128, N], F32, tag="sA", name="pp")
                for it in range(4):
                    nc.tensor.matmul(
                        pp[:, :],
                        _ap(pw1sb, it * 2 * CIN + po * 128,
                            [[4 * 2 * CIN, 128], [1, 128]]),
                        sp[it][:, :],
                        start=(it == 0),
                        stop=(it == 3),
                    )
                a = sb.tile([128, N], F16, tag=f"ac{po}", name=f"ac{po}")
                nc.scalar.activation(
                    a[:, :], pp[:, :], LRELU,
                    bias=pwb1t[:, po:po + 1], alpha=0.01,
                )
                ac.append(a)
            pq = psb.tile([N, PKSL + PBSL], F32, tag="sA", name="pq")
            for it in range(4):
                nc.tensor.matmul(
                    pq[:, 0:PKSL],
                    ac[it][:, :],
                    pwmsb[:, it * 320:it * 320 + PKSL],
                    start=(it == 0),
                    stop=(it == 3),
                )
            for it in range(4):
                nc.tensor.matmul(
                    pq[:, PKSL:PKSL + PBSL],
                    ac[4 + it][:, :],
                    pwmsb[:, it * 320 + PKSL:(it + 1) * 320],
                    start=(it == 0),
                    stop=(it == 3),
                )
            pwt = sb.tile([N, PKSL + PBSL], F16, tag="pwt", name="pwt")
            nc.scalar.activation(pwt[:, :], pq[:, :], COPY)
            nc.sync.dma_start(
                out=_ap(agin[:], BPK, [[BLK, N], [1, PKSL + PBSL]]),
                in_=pwt[:, :],
            )

            # ------------ AllToAll: core c receives, from every rank r,
            # rank r's o-slice of sample c's dynamic weights.
            nc.gpsimd.collective_compute(
                "AllToAll",
                mybir.AluOpType.bypass,
                replica_groups=[list(range(N))],
                ins=[agin[:].opt()],
                outs=[agout[:].opt()],
            )

            # ------------ stage B + stage C, chunk-pipelined
            PBf = []
            S = [sb.tile([128, 9 * 128], F16, tag=f"S{ch}", name=f"S{ch}")
                 for ch in range(4)]
            for ch in range(4):
                # gathers from agout (fp16)
                D = sb.tile([128, 40], F16, tag=f"D{ch}", name=f"D{ch}")
                PKr = sb.tile([128, 8], F16, tag=f"PKr{ch}", name=f"PKr{ch}")
                PBr = sb.tile([128, 8], F16, tag=f"PBr{ch}", name=f"PBr{ch}")
                dpt = _pt(D)
                for hh in range(2):
                    base = (2 * ch + hh) * BLK
                    nc.sync.dma_start(
                        out=_ap(D, 64 * hh * dpt, [[dpt, 64], [4, 9], [1, 4]]),
                        in_=_ap(agout[:], base, [[4, 64], [OSL, 9], [1, 4]]),
                    )
                    nc.sync.dma_start(
                        out=_ap(PKr, 64 * hh * _pt(PKr), [[_pt(PKr), 64], [1, 4]]),
                        in_=_ap(agout[:], base + BPK, [[4, 64], [1, 4]]),
                    )
                    nc.sync.dma_start(
                        out=_ap(PBr, 64 * hh * _pt(PBr), [[_pt(PBr), 64], [1, 1]]),
                        in_=_ap(agout[:], base + BPB, [[1, 64], [1, 1]]),
                    )
                # receiver-side static biases
                nc.vector.tensor_tensor(
                    D[:, 0:36], D[:, 0:36], biasDsb[:, ch * 36:(ch + 1) * 36],
                    op=mybir.AluOpType.add,
                )
                PKb = sb.tile([128, 4], F32, tag=f"PKb{ch}", name=f"PKb{ch}")
                nc.vector.tensor_tensor(
                    PKb[:, :], PKr[:, 0:4], biasPKsb[:, ch * 4:(ch + 1) * 4],
                    op=mybir.AluOpType.add,
                )
                pbf = sb.tile([128, 1], F32, tag=f"PBf{ch}", name=f"PBf{ch}")
                nc.vector.tensor_tensor(
                    pbf[:, :], PBr[:, 0:1], biasPBsb[:, ch:ch + 1],
                    op=mybir.AluOpType.add,
                )
                PBf.append(pbf)
                # W_eff = sum_m PK[:,m] * (perm_m @ D)
                dp = psb.tile([128, 144], F32, tag="sB", name="dp")
                for m2 in range(4):
                    nc.tensor.matmul(
                        dp[:, m2 * 36:(m2 + 1) * 36],
                        permsb[:, m2 * 128:(m2 + 1) * 128],
                        D[:, 0:36],
                        start=True,
                        stop=True,
                    )
                wef = sb.tile([128, 36], F16, tag="wef", name="wef")
                tmp = sb.tile([128, 36], F16, tag="weftmp", name="weftmp")
                nc.vector.tensor_scalar_mul(wef[:, :], dp[:, 0:36], PKb[:, 0:1])
                for m2 in range(1, 4):
                    nc.vector.tensor_scalar_mul(
                        tmp[:, :], dp[:, m2 * 36:(m2 + 1) * 36], PKb[:, m2:m2 + 1]
                    )
                    nc.vector.tensor_add(wef[:, :], wef[:, :], tmp[:, :])
                # expand W_eff -> block-diag S via transpose + select-matmuls
                tpp = psb.tile([36, 128], F16, tag="sB", name="tpp")
                nc.tensor.matmul(
                    tpp[:, :], wef[:, :], identsb[:, :], is_transpose=True,
                    start=True, stop=True,
                )
                wefT = sb.tile([36, 128], F16, tag="wefT", name="wefT")
                nc.vector.tensor_copy(wefT[:, :], tpp[:, :])
                for grp in range(3):
                    sps = psb.tile([128, 3 * 128], F32, tag="sB", name="sps")
                    for tt in range(3):
                        t = grp * 3 + tt
                        nc.tensor.matmul(
                            sps[:, tt * 128:(tt + 1) * 128],
                            selsb[:, t * 128:(t + 1) * 128],
                            wefT[:, :],
                            start=True, stop=True,
                        )
                    nc.vector.tensor_tensor(
                        S[ch][:, grp * 384:(grp + 1) * 384], sps[:, :],
                        masksb[:, :],
                        op=mybir.AluOpType.mult,
                    )

                # ---------- stage C for this chunk
                osb = sbo.tile([128, HW * HW], F16, tag="osb", name="osb")
                for wave in range(4):
                    pcs = [psc.tile([128, 512], F32, tag="pc", name="pc")
                           for _ in range(2)]
                    for tap in range(9):
                        di, dj = tap // 3, tap % 3
                        lhs = S[ch][:, tap * 128:(tap + 1) * 128]
                        for kk, pct in enumerate(pcs):
                            r0 = (wave * 2 + kk) * 8
                            rhs = _ap(xp[ch], (r0 + di) * HWP + dj,
                                      [[XPW, 128], [HWP, 8], [1, HW]])
                            nc.tensor.matmul(
                                pct[:, :],
                                lhs,
                                rhs,
                                start=(tap == 0),
                                stop=(tap == 8),
                            )
                    for kk, pct in enumerate(pcs):
                        s8 = wave * 2 + kk
                        nc.scalar.activation(
                            osb[:, s8 * 512:(s8 + 1) * 512], pct[:, :], IDENT,
                            bias=pbf[:, 0:1],
                        )
                nc.sync.dma_start(
                    out=out[ch * 128:(ch + 1) * 128, :], in_=osb[:, :]
                )

    _split_excess_waits(nc)
    return nc


_NC_CACHE = {}


def _get_nc():
    if "nc" not in _NC_CACHE:
        _NC_CACHE["nc"] = build_nc()
    return _NC_CACHE["nc"]


def make_in_maps(inputs):
    """Host-side shard/layout prep (cast + layout only)."""
    f16 = np.float16
    style = np.asarray(inputs["style_encoding"], np.float32)
    pred = np.asarray(inputs["predicted"], np.float32)
    w1 = np.asarray(inputs["dw1_w"], np.float32).reshape(512, 512)
    w2 = np.asarray(inputs["dw2_w"], np.float32).reshape(2048, 512, 2, 2)
    pk1 = np.asarray(inputs["pk1_w"], np.float32).reshape(512, 512)
    pk2 = np.asarray(inputs["pk2_w"], np.float32).reshape(2048, 512)
    pb1 = np.asarray(inputs["pb1_w"], np.float32).reshape(512, 512)
    pb2 = np.asarray(inputs["pb2_w"], np.float32).reshape(512, 512)
    b1 = np.asarray(inputs["dw1_b"], np.float32)
    b2 = np.asarray(inputs["dw2_b"], np.float32)
    bk1 = np.asarray(inputs["pk1_b"], np.float32)
    bk2 = np.asarray(inputs["pk2_b"], np.float32)
    bb1 = np.asarray(inputs["pb1_b"], np.float32)
    bb2 = np.asarray(inputs["pb2_b"], np.float32)

    # shared tensors
    style_all = np.ascontiguousarray(
        style.transpose(1, 0, 2, 3).reshape(512, N * NPOS)).astype(f16)
    w1t = np.ascontiguousarray(w1.T).astype(f16)
    # fold the 1/16 spatial mean into the first pooled layer's weights
    pw1t = np.ascontiguousarray(
        np.concatenate([pk1.T, pb1.T], axis=1) * (1.0 / NPOS)).astype(f16)
    b1t = np.ascontiguousarray(b1.reshape(4, 128).T).astype(f16)
    pwb1 = np.ascontiguousarray(
        np.concatenate([bk1.reshape(4, 128).T, bb1.reshape(4, 128).T], axis=1)
    ).astype(f16)
    # receiver-side static bias tiles [128, 4*..] (chunk-major free dim)
    biasD = np.broadcast_to(
        b2.reshape(512, 4)[:, None, :], (512, 9, 4)).reshape(512, 36)
    biasD = np.ascontiguousarray(
        biasD.reshape(4, 128, 36).transpose(1, 0, 2).reshape(128, 144)
    ).astype(f16)
    biasPK = np.ascontiguousarray(
        bk2.reshape(512, 4).reshape(4, 128, 4).transpose(1, 0, 2).reshape(128, 16)
    ).astype(f16)
    biasPB = np.ascontiguousarray(bb2.reshape(4, 128).T).astype(f16)

    permm = np.zeros((4, 128, 128), np.float32)
    for m2 in range(4):
        for p in range(128):
            permm[m2, 4 * (p // 4) + m2, p] = 1.0
    permm = np.ascontiguousarray(
        permm.transpose(1, 0, 2).reshape(128, 512)).astype(f16)
    identm = np.eye(128, dtype=np.float32).astype(f16)
    # selm rows indexed k2 = t2*4 + i2 (t-major, matching D's free layout)
    selm = np.zeros((36, 9, 128), np.float32)
    for t in range(9):
        for p in range(128):
            selm[t * 4 + (p % 4), t, p] = 1.0
    selm = selm.reshape(36, 9 * 128).astype(f16)
    maskm = np.zeros((128, 128), np.float32)
    for p in range(128):
        maskm[p, 4 * (p // 4):4 * (p // 4) + 4] = 1.0
    maskm = np.ascontiguousarray(np.tile(maskm, (1, 3))).astype(f16)

    # padded input, per core
    xpad_all = np.pad(pred, ((0, 0), (0, 0), (1, 1), (1, 1)), mode="reflect")
    xpad_all = xpad_all.reshape(N, 512, HWP * HWP).astype(f16)

    in_maps = []
    for c in range(N):
        xz = np.zeros((512, XPW), f16)
        xz[:, :HWP * HWP] = xpad_all[c]
        # dw2 slice, weights-as-moving layout [128, (ib, dy, dx, o)]
        w2s = w2[c * OSL:(c + 1) * OSL]          # [256, 512, 2, 2]
        w2m_ = w2s.transpose(1, 2, 3, 0)         # [512, 2, 2, 256]
        w2m_ = (w2m_.reshape(4, 128, 2, 2, OSL)
                .transpose(1, 0, 2, 3, 4)
                .reshape(128, 16 * OSL))
        # pooled layer-2 moving [128, (it, [pk 256 | pb 64])]
        pk2s = pk2[c * PKSL:(c + 1) * PKSL].T    # [512, 256]
        pb2s = pb2[c * PBSL:(c + 1) * PBSL].T    # [512, 64]
        pwm_ = np.concatenate([pk2s, pb2s], axis=1)   # [512, 320]
        pwm_ = (pwm_.reshape(4, 128, 320)
                .transpose(1, 0, 2)
                .reshape(128, 4 * 320))
        m = {
            "xpad": xz,
            "style": style_all,
            "w1t": w1t,
            "pw1t": pw1t,
            "w2m": np.ascontiguousarray(w2m_).astype(f16),
            "pwm": np.ascontiguousarray(pwm_).astype(f16),
            "b1t": b1t,
            "pwb1": pwb1,
            "biasD": biasD,
            "biasPK": biasPK,
            "biasPB": biasPB,
            "selm": selm,
            "maskm": maskm,
            "permm": permm,
            "ident": identm,
        }
        in_maps.append(m)
    return in_maps


def kernel(**inputs):
    install_profile_shim()
    from concourse.bass_utils import run_bass_kernel_spmd

    nc = _get_nc()
    in_maps = make_in_maps(inputs)
    res = run_bass_kernel_spmd(nc, in_maps, core_ids=list(range(N)))
    outs = [np.asarray(res.results[c]["out"]).reshape(COUT, HW, HW)
            for c in range(N)]
    return np.stack(outs, axis=0).astype(np.float32)


# revision 9
# speedup vs baseline: 1.7441x; 1.0397x over previous
"""AdaConv Trainium2 kernel — 8-core SPMD, data-parallel over batch.

v3: all dynamic tensors fp16; host-packed layouts. Key perf structure:
  * Every dma_start costs ~650ns serial time on the Sync engine, so all
    weights/constants are packed host-side into two [128, W] "wall"
    params (wallA: stage-A layer-1 inputs; wallB: everything else) and
    loaded with ONE dma_start each. xpad (reflect-padded fp16 input) is
    one more. Total dma_start count ~20 vs ~64.
  * Stage A: layer-1 lrelu fused into psum drains (Lrelu activation),
    drained directly into im2col layout for dw2; dw2 weights-as-moving
    -> [72,256] psum -> one merged agin write; pk2/pb2 merged -> [8,320]
    psum -> one agin write. Static biases of sharded layers folded in
    receiver-side from host constants.
  * AllToAll redistributes per-sample dynamic weights (fp16, 42KB).
  * Stage B: per chunk, gather D/PK/PB (4-D AP DMAs), build block-diag
    fp16 stationaries S[ch] via perm-matmuls + scales + select-matmuls.
  * Stage C: grouped 3x3 conv, 9 psum-accumulated fp16 matmuls per
    2-sub wave, 4 rotating psum banks, fp16 output (upcast on host).
"""
import sys
import types

sys.path.insert(0, "/opt/trn_rl_repo")

import numpy as np

import concourse.bass as bass
import concourse.mybir as mybir

N = 8          # batch == cores
CIN = 512
COUT = 512
HW = 64        # spatial
HWP = 66       # padded
XPW = 4384     # per-chunk padded width (66*66=4356 used)
NPOS = 16      # style spatial 4x4
OSL = 2048 // N      # dw2 out-channel slice per core (256)
PKSL = 2048 // N     # pk2 slice (256)
PBSL = 512 // N      # pb2 slice (64)
# AllToAll per-rank block (fp16 elems): [dw 9*256 | pk 256 | pb 64]
BPK = 9 * OSL              # 2304
BPB = BPK + PKSL           # 2560
BLK = BPB + PBSL           # 2624
AG_SZ = N * BLK            # 20992

# wallA layout (cols): st4 | w1 | b1t
CA_ST = 0
CA_W1 = 512
CA_B1 = CA_W1 + 2048
WA = CA_B1 + 4            # 2564
# wallB layout (cols)
CB_PW1 = 0
CB_PWB1 = CB_PW1 + 4096
CB_W2M = CB_PWB1 + 8
CB_PWM = CB_W2M + 4096
CB_PERM = CB_PWM + 1280
CB_IDENT = CB_PERM + 512
CB_SEL = CB_IDENT + 128
CB_MASK = CB_SEL + 1152
CB_BIASD = CB_MASK + 384
CB_BIASPK = CB_BIASD + 144
CB_BIASPB = CB_BIASPK + 16
WB = CB_BIASPB + 4        # 11824

F32 = mybir.dt.float32
F16 = mybir.dt.float16


# ---------------------------------------------------------------- tile patch
def _install_tile_patch():
    """walrus here rejects Drain instructions with >1 sync-wait; spread the
    Tile tail-drain waits over individual SP nops."""
    import concourse.tile as tile_mod
    from concourse.vector_clock import ScopedClock

    def _patched(self, tick_clock, wait_clock):
        nc = self.nc
        drain_inst = nc.sync.drain()
        wait_clock.add_sem_waits(
            drain_inst.ins, ScopedClock({None: tick_clock.global_clock})
        )
        waits = list(drain_inst.ins.sync_info.on_wait or [])
        if len(waits) > 1:
            drain_inst.ins.sync_info.on_wait = waits[:1]
            for w in waits[1:]:
                nop = nc.sync.nop(nofuse=True, hint="tail_wait_split")
                if nop.ins.sync_info is None:
                    nop.ins.sync_info = mybir.SyncInfo(on_wait=[w], on_update=[])
                else:
                    nop.ins.sync_info.on_wait = [w]
        nc.all_engine_barrier()
        assert self.sems is not None
        popped = nc._tile_sem_poison_stack.pop()
        assert popped is self._sem_poison
        nc.clear_and_free_semaphores(list(self.sems.allocated().values()))
        nc.all_engine_barrier()

    tile_mod.TileContext._drain_and_barrier = _patched


_install_tile_patch()
from concourse.tile import TileContext  # noqa: E402


def install_profile_shim():
    """antenv.axon_hooks is missing from this image; recreate it so
    run_bass_kernel_spmd(trace=True) can capture NTFF profiles."""
    if "antenv.axon_hooks" in sys.modules:
        return
    import antenv

    mod = types.ModuleType("antenv.axon_hooks")
    mod._hook = None
    mod.set_axon_ntff_profile_hook = lambda h: setattr(mod, "_hook", h)
    mod.get_axon_ntff_profile_hook = lambda: mod._hook
    sys.modules["antenv.axon_hooks"] = mod
    antenv.axon_hooks = mod
    try:
        if "/root/.axon_site" not in sys.path:
            sys.path.insert(0, "/root/.axon_site")
        from trn_agent_boot.trn_boot import _ntff_profile_via_ctypes

        hook = _ntff_profile_via_ctypes("/opt/axon/libaxon_pjrt.so")
        mod.set_axon_ntff_profile_hook(hook)
    except Exception:
        pass


def _ap(t_ap, offset, dims):
    """Custom flat AP over a tile's underlying tensor."""
    return bass.AP(t_ap.tensor, offset, [list(d) for d in dims])


def _pt(t):
    """Physical partition pitch (elements) of a tile."""
    return t[:, :].ap[0][0]


def _split_excess_waits(nc, max_waits=1):
    """This walrus build rejects instructions carrying more than ~1 sync-wait.
    Move excess waits onto same-engine NoOps inserted just before."""
    n_split = 0
    for f in nc.m.functions:
        for bb in f.blocks:
            newlist = []
            for inst in bb.instructions:
                si = getattr(inst, "sync_info", None)
                if si is not None and si.on_wait and len(si.on_wait) > max_waits:
                    waits = list(si.on_wait)
                    for k, w in enumerate(waits[max_waits:]):
                        nop = mybir.InstNoOp(
                            name=f"{inst.name}_ws{k}",
                            engine=inst.engine,
                            bass_nofuse=True,
                            sync_info=mybir.SyncInfo(on_wait=[w], on_update=[]),
                        )
                        newlist.append(nop)
                        n_split += 1
                    si.on_wait = waits[:max_waits]
                newlist.append(inst)
            try:
                bb.instructions[:] = newlist
            except TypeError:
                bb.set_instructions(newlist)
    return n_split


LRELU = mybir.ActivationFunctionType.Lrelu
IDENT = mybir.ActivationFunctionType.Identity
COPY = mybir.ActivationFunctionType.Copy


def build_nc():
    nc = bass.Bass(target_bir_lowering=False)

    wallA = nc.declare_dram_parameter("wallA", [128, WA], F16, isOutput=False)
    wallB = nc.declare_dram_parameter("wallB", [128, WB], F16, isOutput=False)
    xpad = nc.declare_dram_parameter("xpad", [128, 4 * XPW], F16, isOutput=False)
    out = nc.declare_dram_parameter("out", [COUT, HW * HW], F16, isOutput=True)

    with TileContext(nc) as tc:
        with (
            tc.tile_pool(name="sb", bufs=1) as sb,
            tc.tile_pool(name="sbx", bufs=1) as sbx,
            tc.tile_pool(name="sbo", bufs=2) as sbo,
            tc.tile_pool(name="psb", bufs=2, space="PSUM") as psb,
            tc.tile_pool(name="psc", bufs=4, space="PSUM") as psc,
            tc.tile_pool(name="dram", bufs=1, space="DRAM") as dram,
        ):
            wa = sb.tile([128, WA], F16, tag="wa", name="wa")
            nc.sync.dma_start(out=wa[:, :], in_=wallA[:, :])
            wb = sb.tile([128, WB], F16, tag="wb", name="wb")
            nc.sync.dma_start(out=wb[:, :], in_=wallB[:, :])
            xpx = sbx.tile([128, 4 * XPW], F16, tag="xpx", name="xpx")
            nc.sync.dma_start(out=xpx[:, :], in_=xpad[:, :])
            wap = _pt(wa)
            wbp = _pt(wb)

            def wA(col, np_, nf):
                return _ap(wa, col, [[wap, np_], [1, nf]])

            def wB(col, np_, nf):
                return _ap(wb, col, [[wbp, np_], [1, nf]])

            # ------------ stage A: h = lrelu(W1 s + b1), drained directly
            # into im2col layout h2[ot][:, dydx*72:+72] = (n, ty, tx) windows
            h2 = [sb.tile([128, 4 * 72], F16, tag=f"h2{ot}", name=f"h2{ot}")
                  for ot in range(4)]
            for ot in range(4):
                pa = psb.tile([128, 128], F32, tag="sA", name="pa")
                for it in range(4):
                    nc.tensor.matmul(
                        pa[:, :],
                        wA(CA_W1 + it * CIN + ot * 128, 128, 128),
                        wA(CA_ST + it * 128, 128, 128),
                        start=(it == 0),
                        stop=(it == 3),
                    )
                pap = _pt(pa)
                for dy in range(2):
                    for dx in range(2):
                        nc.scalar.activation(
                            h2[ot][:, (dy * 2 + dx) * 72:(dy * 2 + dx + 1) * 72],
                            _ap(pa, dy * 4 + dx,
                                [[pap, 128], [NPOS, N], [4, 3], [1, 3]]),
                            LRELU,
                            bias=wA(CA_B1 + ot, 128, 1), alpha=0.01,
                        )

            # ------------ stage A: dw2 slice, weights-as-moving
            # psum [72=(n,ty,tx), 256=o-slice]
            agin = dram.tile([AG_SZ], F16)
            agout = dram.tile([AG_SZ], F16)
            pd = psb.tile([72, OSL], F32, tag="sA", name="pd")
            k = 0
            for ib in range(4):
                for dydx in range(4):
                    nc.tensor.matmul(
                        pd[:, :],
                        h2[ib][:, dydx * 72:(dydx + 1) * 72],
                        wB(CB_W2M + (ib * 4 + dydx) * OSL, 128, OSL),
                        start=(k == 0),
                        stop=(k == 15),
                    )
                    k += 1
            dwt = sb.tile([72, OSL], F16, tag="dwt", name="dwt")
            nc.scalar.activation(dwt[:, :], pd[:, :], COPY)
            for n in range(N):
                nc.sync.dma_start(
                    out=_ap(agin[:], n * BLK, [[OSL, 9], [1, OSL]]),
                    in_=dwt[9 * n:9 * n + 9, :],
                )

            # ------------ stage A: pooled path (pk1|pb1 then pk2|pb2)
            sp = [sb.tile([128, N], F16, tag=f"sp{i}", name=f"sp{i}")
                  for i in range(4)]
            with nc.allow_low_precision("16-term style pool in fp16"):
                for i in range(4):
                    nc.vector.tensor_reduce(
                        sp[i][:, :],
                        _ap(wa, CA_ST + i * 128,
                            [[wap, 128], [NPOS, N], [1, NPOS]]),
                        axis=mybir.AxisListType.X,
                        op=mybir.AluOpType.add,
                    )
            ac = []
            for po in range(8):
                pp = psb.tile([128, N], F32, tag="sA", name="pp")
                for it in range(4):
                    nc.tensor.matmul(
                        pp[:, :],
                        wB(CB_PW1 + it * 2 * CIN + po * 128, 128, 128),
                        sp[it][:, :],
                        start=(it == 0),
                        stop=(it == 3),
                    )
                a = sb.tile([128, N], F16, tag=f"ac{po}", name=f"ac{po}")
                nc.scalar.activation(
                    a[:, :], pp[:, :], LRELU,
                    bias=wB(CB_PWB1 + po, 128, 1), alpha=0.01,
                )
                ac.append(a)
            pq = psb.tile([N, PKSL + PBSL], F32, tag="sA", name="pq")
            for it in range(4):
                nc.tensor.matmul(
                    pq[:, 0:PKSL],
                    ac[it][:, :],
                    wB(CB_PWM + it * 320, 128, PKSL),
                    start=(it == 0),
                    stop=(it == 3),
                )
            for it in range(4):
                nc.tensor.matmul(
                    pq[:, PKSL:PKSL + PBSL],
                    ac[4 + it][:, :],
                    wB(CB_PWM + it * 320 + PKSL, 128, PBSL),
                    start=(it == 0),
                    stop=(it == 3),
                )
            pwt = sb.tile([N, PKSL + PBSL], F16, tag="pwt", name="pwt")
            nc.scalar.activation(pwt[:, :], pq[:, :], COPY)
            nc.sync.dma_start(
                out=_ap(agin[:], BPK, [[BLK, N], [1, PKSL + PBSL]]),
                in_=pwt[:, :],
            )

            # ------------ AllToAll: core c receives, from every rank r,
            # rank r's o-slice of sample c's dynamic weights.
            nc.gpsimd.collective_compute(
                "AllToAll",
                mybir.AluOpType.bypass,
                replica_groups=[list(range(N))],
                ins=[agin[:].opt()],
                outs=[agout[:].opt()],
            )

            # ------------ stage B + stage C, chunk-pipelined
            S = [sb.tile([128, 9 * 128], F16, tag=f"S{ch}", name=f"S{ch}")
                 for ch in range(4)]
            for ch in range(4):
                # gathers from agout (fp16), merged 4-D APs
                D = sb.tile([128, 40], F16, tag=f"D{ch}", name=f"D{ch}")
                PKr = sb.tile([128, 8], F16, tag=f"PKr{ch}", name=f"PKr{ch}")
                PBr = sb.tile([128, 8], F16, tag=f"PBr{ch}", name=f"PBr{ch}")
                dpt = _pt(D)
                for hh in range(2):
                    base = (2 * ch + hh) * BLK
                    nc.sync.dma_start(
                        out=_ap(D, 64 * hh * dpt, [[dpt, 64], [4, 9], [1, 4]]),
                        in_=_ap(agout[:], base, [[4, 64], [OSL, 9], [1, 4]]),
                    )
                    nc.sync.dma_start(
                        out=_ap(PKr, 64 * hh * _pt(PKr),
                                [[_pt(PKr), 64], [1, 4]]),
                        in_=_ap(agout[:], base + BPK, [[4, 64], [1, 4]]),
                    )
                    nc.sync.dma_start(
                        out=_ap(PBr, 64 * hh * _pt(PBr),
                                [[_pt(PBr), 64], [1, 1]]),
                        in_=_ap(agout[:], base + BPB, [[1, 64], [1, 1]]),
                    )
                # receiver-side static biases
                nc.vector.tensor_tensor(
                    D[:, 0:36], D[:, 0:36],
                    wB(CB_BIASD + ch * 36, 128, 36),
                    op=mybir.AluOpType.add,
                )
                PKb = sb.tile([128, 4], F32, tag=f"PKb{ch}", name=f"PKb{ch}")
                nc.vector.tensor_tensor(
                    PKb[:, :], PKr[:, 0:4],
                    wB(CB_BIASPK + ch * 4, 128, 4),
                    op=mybir.AluOpType.add,
                )
                pbf = sb.tile([128, 1], F32, tag=f"PBf{ch}", name=f"PBf{ch}")
                nc.vector.tensor_tensor(
                    pbf[:, :], PBr[:, 0:1],
                    wB(CB_BIASPB + ch, 128, 1),
                    op=mybir.AluOpType.add,
                )
                # W_eff = sum_m PK[:,m] * (perm_m @ D)
                dp = psb.tile([128, 144], F32, tag="sB", name="dp")
                for m2 in range(4):
                    nc.tensor.matmul(
                        dp[:, m2 * 36:(m2 + 1) * 36],
                        wB(CB_PERM + m2 * 128, 128, 128),
                        D[:, 0:36],
                        start=True,
                        stop=True,
                    )
                wef = sb.tile([128, 36], F16, tag="wef", name="wef")
                tmp = sb.tile([128, 36], F16, tag="weftmp", name="weftmp")
                nc.vector.tensor_scalar_mul(wef[:, :], dp[:, 0:36], PKb[:, 0:1])
                for m2 in range(1, 4):
                    nc.vector.tensor_scalar_mul(
                        tmp[:, :], dp[:, m2 * 36:(m2 + 1) * 36], PKb[:, m2:m2 + 1]
                    )
                    nc.vector.tensor_add(wef[:, :], wef[:, :], tmp[:, :])
                # expand W_eff -> block-diag S via transpose + select-matmuls
                tpp = psb.tile([36, 128], F16, tag="sB", name="tpp")
                nc.tensor.matmul(
                    tpp[:, :], wef[:, :], wB(CB_IDENT, 128, 128),
                    is_transpose=True, start=True, stop=True,
                )
                wefT = sb.tile([36, 128], F16, tag="wefT", name="wefT")
                nc.vector.tensor_copy(wefT[:, :], tpp[:, :])
                for grp in range(3):
                    sps = psb.tile([128, 3 * 128], F32, tag="sB", name="sps")
                    for tt in range(3):
                        t = grp * 3 + tt
                        nc.tensor.matmul(
                            sps[:, tt * 128:(tt + 1) * 128],
                            wB(CB_SEL + t * 128, 36, 128),
                            wefT[:, :],
                            start=True, stop=True,
                        )
                    nc.vector.tensor_tensor(
                        S[ch][:, grp * 384:(grp + 1) * 384], sps[:, :],
                        wB(CB_MASK, 128, 384),
                        op=mybir.AluOpType.mult,
                    )

                # ---------- stage C for this chunk
                osb = sbo.tile([128, HW * HW], F16, tag="osb", name="osb")
                for wave in range(4):
                    pcs = [psc.tile([128, 512], F32, tag="pc", name="pc")
                           for _ in range(2)]
                    for tap in range(9):
                        di, dj = tap // 3, tap % 3
                        lhs = S[ch][:, tap * 128:(tap + 1) * 128]
                        for kk, pct in enumerate(pcs):
                            r0 = (wave * 2 + kk) * 8
                            rhs = _ap(xpx, ch * XPW + (r0 + di) * HWP + dj,
                                      [[4 * XPW, 128], [HWP, 8], [1, HW]])
                            nc.tensor.matmul(
                                pct[:, :],
                                lhs,
                                rhs,
                                start=(tap == 0),
                                stop=(tap == 8),
                            )
                    for kk, pct in enumerate(pcs):
                        s8 = wave * 2 + kk
                        nc.scalar.activation(
                            osb[:, s8 * 512:(s8 + 1) * 512], pct[:, :], IDENT,
                            bias=pbf[:, 0:1],
                        )
                nc.sync.dma_start(
                    out=out[ch * 128:(ch + 1) * 128, :], in_=osb[:, :]
                )

    _split_excess_waits(nc)
    return nc


_NC_CACHE = {}


def _get_nc():
    if "nc" not in _NC_CACHE:
        _NC_CACHE["nc"] = build_nc()
    return _NC_CACHE["nc"]


def _pack128(arr):
    """[512, X] -> [128, 4*X] with free idx = blk*X + x."""
    xw = arr.shape[1]
    return np.ascontiguousarray(
        arr.reshape(4, 128, xw).transpose(1, 0, 2).reshape(128, 4 * xw))


def make_in_maps(inputs):
    """Host-side shard/layout prep (cast + layout only)."""
    f16 = np.float16
    style = np.asarray(inputs["style_encoding"], np.float32)
    pred = np.asarray(inputs["predicted"], np.float32)
    w1 = np.asarray(inputs["dw1_w"], np.float32).reshape(512, 512)
    w2 = np.asarray(inputs["dw2_w"], np.float32).reshape(2048, 512, 2, 2)
    pk1 = np.asarray(inputs["pk1_w"], np.float32).reshape(512, 512)
    pk2 = np.asarray(inputs["pk2_w"], np.float32).reshape(2048, 512)
    pb1 = np.asarray(inputs["pb1_w"], np.float32).reshape(512, 512)
    pb2 = np.asarray(inputs["pb2_w"], np.float32).reshape(512, 512)
    b1 = np.asarray(inputs["dw1_b"], np.float32)
    b2 = np.asarray(inputs["dw2_b"], np.float32)
    bk1 = np.asarray(inputs["pk1_b"], np.float32)
    bk2 = np.asarray(inputs["pk2_b"], np.float32)
    bb1 = np.asarray(inputs["pb1_b"], np.float32)
    bb2 = np.asarray(inputs["pb2_b"], np.float32)

    # ---- wallA (shared): st4 | w1 | b1t
    st4 = _pack128(style.transpose(1, 0, 2, 3).reshape(512, N * NPOS))
    w1p = _pack128(np.ascontiguousarray(w1.T))
    b1t = b1.reshape(4, 128).T
    wallA = np.concatenate([st4, w1p, b1t], axis=1).astype(f16)
    assert wallA.shape[1] == WA, wallA.shape

    # ---- wallB (per-core only in w2m/pwm slices)
    # fold the 1/16 spatial mean into the first pooled layer's weights
    pw1p = _pack128(np.ascontiguousarray(
        np.concatenate([pk1.T, pb1.T], axis=1) * (1.0 / NPOS)))
    pwb1 = np.concatenate(
        [bk1.reshape(4, 128).T, bb1.reshape(4, 128).T], axis=1)
    permm = np.zeros((4, 128, 128), np.float32)
    for m2 in range(4):
        for p in range(128):
            permm[m2, 4 * (p // 4) + m2, p] = 1.0
    permm = permm.transpose(1, 0, 2).reshape(128, 512)
    identm = np.eye(128, dtype=np.float32)
    # selm rows k2 = t2*4 + i2 (t-major, matching D's free layout), padded
    selm = np.zeros((36, 9, 128), np.float32)
    for t in range(9):
        for p in range(128):
            selm[t * 4 + (p % 4), t, p] = 1.0
    selm = np.concatenate(
        [selm.reshape(36, 9 * 128), np.zeros((92, 9 * 128), np.float32)], 0)
    maskm = np.zeros((128, 128), np.float32)
    for p in range(128):
        maskm[p, 4 * (p // 4):4 * (p // 4) + 4] = 1.0
    maskm = np.tile(maskm, (1, 3))
    biasD = np.broadcast_to(
        b2.reshape(512, 4)[:, None, :], (512, 9, 4)).reshape(512, 36)
    biasD = _pack128(biasD)
    biasPK = _pack128(bk2.reshape(512, 4))
    biasPB = bb2.reshape(4, 128).T

    # padded input, per core
    xpad_all = np.pad(pred, ((0, 0), (0, 0), (1, 1), (1, 1)), mode="reflect")
    xpad_all = xpad_all.reshape(N, 512, HWP * HWP).astype(f16)

    in_maps = []
    for c in range(N):
        xz = np.zeros((512, XPW), f16)
        xz[:, :HWP * HWP] = xpad_all[c]
        xz = np.ascontiguousarray(
            xz.reshape(4, 128, XPW).transpose(1, 0, 2).reshape(128, 4 * XPW))
        # dw2 slice, weights-as-moving layout [128, (ib, dydx, o)]
        w2s = w2[c * OSL:(c + 1) * OSL]          # [256, 512, 2, 2]
        w2m_ = w2s.transpose(1, 2, 3, 0)         # [512, 2, 2, 256]
        w2m_ = (w2m_.reshape(4, 128, 2, 2, OSL)
                .transpose(1, 0, 2, 3, 4)
                .reshape(128, 16 * OSL))
        # pooled layer-2 moving [128, (it, [pk 256 | pb 64])]
        pk2s = pk2[c * PKSL:(c + 1) * PKSL].T    # [512, 256]
        pb2s = pb2[c * PBSL:(c + 1) * PBSL].T    # [512, 64]
        pwm_ = np.concatenate([pk2s, pb2s], axis=1)   # [512, 320]
        pwm_ = (pwm_.reshape(4, 128, 320)
                .transpose(1, 0, 2)
                .reshape(128, 4 * 320))
        wallB = np.concatenate(
            [pw1p, pwb1, w2m_, pwm_, permm, identm, selm, maskm,
             biasD, biasPK, biasPB], axis=1).astype(f16)
        assert wallB.shape[1] == WB, wallB.shape
        in_maps.append({
            "wallA": wallA,
            "wallB": np.ascontiguousarray(wallB),
            "xpad": xz,
        })
    return in_maps


def kernel(**inputs):
    install_profile_shim()
    from concourse.bass_utils import run_bass_kernel_spmd

    nc = _get_nc()
    in_maps = make_in_maps(inputs)
    res = run_bass_kernel_spmd(nc, in_maps, core_ids=list(range(N)))
    outs = [np.asarray(res.results[c]["out"]).reshape(COUT, HW, HW)
            for c in range(N)]
    return np.stack(outs, axis=0).astype(np.float32)


# revision 12
# speedup vs baseline: 1.8725x; 1.0736x over previous
"""AdaConv Trainium2 kernel — 8-core SPMD, data-parallel over batch.

v3: all dynamic tensors fp16; host-packed layouts. Key perf structure:
  * Every dma_start costs ~650ns serial time on the Sync engine, so all
    weights/constants are packed host-side into two [128, W] "wall"
    params (wallA: stage-A layer-1 inputs; wallB: everything else) and
    loaded with ONE dma_start each. xpad (reflect-padded fp16 input) is
    one more. Total dma_start count ~20 vs ~64.
  * Stage A: layer-1 lrelu fused into psum drains (Lrelu activation),
    drained directly into im2col layout for dw2; dw2 weights-as-moving
    -> [72,256] psum -> one merged agin write; pk2/pb2 merged -> [8,320]
    psum -> one agin write. Static biases of sharded layers folded in
    receiver-side from host constants.
  * AllToAll redistributes per-sample dynamic weights (fp16, 42KB).
  * Stage B: per chunk, gather D/PK/PB (4-D AP DMAs), build block-diag
    fp16 stationaries S[ch] via perm-matmuls + scales + select-matmuls.
  * Stage C: grouped 3x3 conv, 9 psum-accumulated fp16 matmuls per
    2-sub wave, 4 rotating psum banks, fp16 output (upcast on host).
"""
import sys
import types

sys.path.insert(0, "/opt/trn_rl_repo")

import numpy as np

import concourse.bass as bass
import concourse.mybir as mybir

N = 8          # batch == cores
CIN = 512
COUT = 512
HW = 64        # spatial
HWP = 66       # padded
XPW = 4384     # per-chunk padded width (66*66=4356 used)
NPOS = 16      # style spatial 4x4
OSL = 2048 // N      # dw2 out-channel slice per core (256)
PKSL = 2048 // N     # pk2 slice (256)
PBSL = 512 // N      # pb2 slice (64)
# AllToAll per-rank block (fp16 elems): [dw 9*256 | pk 256 | pb 64]
BPK = 9 * OSL              # 2304
BPB = BPK + PKSL           # 2560
BLK = BPB + PBSL           # 2624
AG_SZ = N * BLK            # 20992

# wallA layout (cols): st4 | w1 | b1t
CA_ST = 0
CA_W1 = 512
CA_B1 = CA_W1 + 2048
WA = CA_B1 + 4            # 2564
# wallB1 layout (cols): stage-A layer-2 weights
CB_PW1 = 0
CB_PWB1 = CB_PW1 + 4096
CB_W2M = CB_PWB1 + 8
CB_PWM = CB_W2M + 4096
WB1 = CB_PWM + 1280       # 9480
# wallB2 layout (cols): stage-B constants
C2_PERM = 0
C2_IDENT = C2_PERM + 512
C2_SEL = C2_IDENT + 128
C2_MASK = C2_SEL + 1152
C2_BIASD = C2_MASK + 384
C2_BIASPK = C2_BIASD + 144
C2_BIASPB = C2_BIASPK + 16
WB2 = C2_BIASPB + 4       # 2340

F32 = mybir.dt.float32
F16 = mybir.dt.float16


# ---------------------------------------------------------------- tile patch
def _install_tile_patch():
    """walrus here rejects Drain instructions with >1 sync-wait; spread the
    Tile tail-drain waits over individual SP nops."""
    import concourse.tile as tile_mod
    from concourse.vector_clock import ScopedClock

    def _patched(self, tick_clock, wait_clock):
        nc = self.nc
        drain_inst = nc.sync.drain()
        wait_clock.add_sem_waits(
            drain_inst.ins, ScopedClock({None: tick_clock.global_clock})
        )
        waits = list(drain_inst.ins.sync_info.on_wait or [])
        if len(waits) > 1:
            drain_inst.ins.sync_info.on_wait = waits[:1]
            for w in waits[1:]:
                nop = nc.sync.nop(nofuse=True, hint="tail_wait_split")
                if nop.ins.sync_info is None:
                    nop.ins.sync_info = mybir.SyncInfo(on_wait=[w], on_update=[])
                else:
                    nop.ins.sync_info.on_wait = [w]
        nc.all_engine_barrier()
        assert self.sems is not None
        popped = nc._tile_sem_poison_stack.pop()
        assert popped is self._sem_poison
        nc.clear_and_free_semaphores(list(self.sems.allocated().values()))
        nc.all_engine_barrier()

    tile_mod.TileContext._drain_and_barrier = _patched


_install_tile_patch()
from concourse.tile import TileContext  # noqa: E402


def install_profile_shim():
    """antenv.axon_hooks is missing from this image; recreate it so
    run_bass_kernel_spmd(trace=True) can capture NTFF profiles."""
    if "antenv.axon_hooks" in sys.modules:
        return
    import antenv

    mod = types.ModuleType("antenv.axon_hooks")
    mod._hook = None
    mod.set_axon_ntff_profile_hook = lambda h: setattr(mod, "_hook", h)
    mod.get_axon_ntff_profile_hook = lambda: mod._hook
    sys.modules["antenv.axon_hooks"] = mod
    antenv.axon_hooks = mod
    try:
        if "/root/.axon_site" not in sys.path:
            sys.path.insert(0, "/root/.axon_site")
        from trn_agent_boot.trn_boot import _ntff_profile_via_ctypes

        hook = _ntff_profile_via_ctypes("/opt/axon/libaxon_pjrt.so")
        mod.set_axon_ntff_profile_hook(hook)
    except Exception:
        pass


def _ap(t_ap, offset, dims):
    """Custom flat AP over a tile's underlying tensor."""
    return bass.AP(t_ap.tensor, offset, [list(d) for d in dims])


def _pt(t):
    """Physical partition pitch (elements) of a tile."""
    return t[:, :].ap[0][0]


def _split_excess_waits(nc, max_waits=1):
    """This walrus build rejects instructions carrying more than ~1 sync-wait.
    Move excess waits onto same-engine NoOps inserted just before."""
    n_split = 0
    for f in nc.m.functions:
        for bb in f.blocks:
            newlist = []
            for inst in bb.instructions:
                si = getattr(inst, "sync_info", None)
                if si is not None and si.on_wait and len(si.on_wait) > max_waits:
                    waits = list(si.on_wait)
                    for k, w in enumerate(waits[max_waits:]):
                        nop = mybir.InstNoOp(
                            name=f"{inst.name}_ws{k}",
                            engine=inst.engine,
                            bass_nofuse=True,
                            sync_info=mybir.SyncInfo(on_wait=[w], on_update=[]),
                        )
                        newlist.append(nop)
                        n_split += 1
                    si.on_wait = waits[:max_waits]
                newlist.append(inst)
            try:
                bb.instructions[:] = newlist
            except TypeError:
                bb.set_instructions(newlist)
    return n_split


LRELU = mybir.ActivationFunctionType.Lrelu
IDENT = mybir.ActivationFunctionType.Identity
COPY = mybir.ActivationFunctionType.Copy


def build_nc():
    nc = bass.Bass(target_bir_lowering=False)

    wallA = nc.declare_dram_parameter("wallA", [128, WA], F16, isOutput=False)
    wallB1 = nc.declare_dram_parameter("wallB1", [128, WB1], F16, isOutput=False)
    wallB2 = nc.declare_dram_parameter("wallB2", [128, WB2], F16, isOutput=False)
    xpad = nc.declare_dram_parameter("xpad", [128, 4 * XPW], F16, isOutput=False)
    out = nc.declare_dram_parameter("out", [COUT, HW * HW], F16, isOutput=True)

    with TileContext(nc) as tc:
        with (
            tc.tile_pool(name="sb", bufs=1) as sb,
            tc.tile_pool(name="sbx", bufs=1) as sbx,
            tc.tile_pool(name="sbo", bufs=2) as sbo,
            tc.tile_pool(name="psb", bufs=2, space="PSUM") as psb,
            tc.tile_pool(name="psc", bufs=4, space="PSUM") as psc,
            tc.tile_pool(name="dram", bufs=1, space="DRAM") as dram,
        ):
            wa = sb.tile([128, WA], F16, tag="wa", name="wa")
            nc.sync.dma_start(out=wa[:, :], in_=wallA[:, :])
            wb = sb.tile([128, WB1], F16, tag="wb", name="wb")
            nc.sync.dma_start(out=wb[:, :], in_=wallB1[:, :])
            wc = sb.tile([128, WB2], F16, tag="wc", name="wc")
            nc.sync.dma_start(out=wc[:, :], in_=wallB2[:, :])
            xpx = sbx.tile([128, 4 * XPW], F16, tag="xpx", name="xpx")
            nc.sync.dma_start(out=xpx[:, :], in_=xpad[:, :])
            wap = _pt(wa)
            wbp = _pt(wb)

            def wA(col, np_, nf):
                return _ap(wa, col, [[wap, np_], [1, nf]])

            def wB(col, np_, nf):
                return _ap(wb, col, [[wbp, np_], [1, nf]])

            wcp = _pt(wc)

            def wC(col, np_, nf):
                return _ap(wc, col, [[wcp, np_], [1, nf]])

            # ------------ stage A: h = lrelu(W1 s + b1), drained directly
            # into im2col layout h2[ot][:, dydx*72:+72] = (n, ty, tx) windows
            h2 = [sb.tile([128, 4 * 72], F16, tag=f"h2{ot}", name=f"h2{ot}")
                  for ot in range(4)]
            for ot in range(4):
                pa = psb.tile([128, 128], F32, tag="sA", name="pa")
                for it in range(4):
                    nc.tensor.matmul(
                        pa[:, :],
                        wA(CA_W1 + it * CIN + ot * 128, 128, 128),
                        wA(CA_ST + it * 128, 128, 128),
                        start=(it == 0),
                        stop=(it == 3),
                    )
                pap = _pt(pa)
                for dy in range(2):
                    for dx in range(2):
                        nc.scalar.activation(
                            h2[ot][:, (dy * 2 + dx) * 72:(dy * 2 + dx + 1) * 72],
                            _ap(pa, dy * 4 + dx,
                                [[pap, 128], [NPOS, N], [4, 3], [1, 3]]),
                            LRELU,
                            bias=wA(CA_B1 + ot, 128, 1), alpha=0.01,
                        )

            # ------------ stage A: dw2 slice, weights-as-moving
            # psum [72=(n,ty,tx), 256=o-slice]
            agin = dram.tile([AG_SZ], F16)
            agout = dram.tile([AG_SZ], F16)
            pd = psb.tile([72, OSL], F32, tag="sA", name="pd")
            k = 0
            for ib in range(4):
                for dydx in range(4):
                    nc.tensor.matmul(
                        pd[:, :],
                        h2[ib][:, dydx * 72:(dydx + 1) * 72],
                        wB(CB_W2M + (ib * 4 + dydx) * OSL, 128, OSL),
                        start=(k == 0),
                        stop=(k == 15),
                    )
                    k += 1
            dwt = sb.tile([72, OSL], F16, tag="dwt", name="dwt")
            nc.scalar.activation(dwt[:, :], pd[:, :], COPY)
            for n in range(N):
                eng = nc.scalar if n < 4 else nc.gpsimd
                eng.dma_start(
                    out=_ap(agin[:], n * BLK, [[OSL, 9], [1, OSL]]),
                    in_=dwt[9 * n:9 * n + 9, :],
                )

            # ------------ stage A: pooled path (pk1|pb1 then pk2|pb2)
            sp = [sb.tile([128, N], F16, tag=f"sp{i}", name=f"sp{i}")
                  for i in range(4)]
            with nc.allow_low_precision("16-term style pool in fp16"):
                for i in range(4):
                    nc.vector.tensor_reduce(
                        sp[i][:, :],
                        _ap(wa, CA_ST + i * 128,
                            [[wap, 128], [NPOS, N], [1, NPOS]]),
                        axis=mybir.AxisListType.X,
                        op=mybir.AluOpType.add,
                    )
            ac = []
            for po in range(8):
                pp = psb.tile([128, N], F32, tag="sA", name="pp")
                for it in range(4):
                    nc.tensor.matmul(
                        pp[:, :],
                        wB(CB_PW1 + it * 2 * CIN + po * 128, 128, 128),
                        sp[it][:, :],
                        start=(it == 0),
                        stop=(it == 3),
                    )
                a = sb.tile([128, N], F16, tag=f"ac{po}", name=f"ac{po}")
                nc.scalar.activation(
                    a[:, :], pp[:, :], LRELU,
                    bias=wB(CB_PWB1 + po, 128, 1), alpha=0.01,
                )
                ac.append(a)
            pq = psb.tile([N, PKSL + PBSL], F32, tag="sA", name="pq")
            for it in range(4):
                nc.tensor.matmul(
                    pq[:, 0:PKSL],
                    ac[it][:, :],
                    wB(CB_PWM + it * 320, 128, PKSL),
                    start=(it == 0),
                    stop=(it == 3),
                )
            for it in range(4):
                nc.tensor.matmul(
                    pq[:, PKSL:PKSL + PBSL],
                    ac[4 + it][:, :],
                    wB(CB_PWM + it * 320 + PKSL, 128, PBSL),
                    start=(it == 0),
                    stop=(it == 3),
                )
            pwt = sb.tile([N, PKSL + PBSL], F16, tag="pwt", name="pwt")
            nc.scalar.activation(pwt[:, :], pq[:, :], COPY)
            nc.gpsimd.dma_start(
                out=_ap(agin[:], BPK, [[BLK, N], [1, PKSL + PBSL]]),
                in_=pwt[:, :],
            )

            # ------------ AllToAll: core c receives, from every rank r,
            # rank r's o-slice of sample c's dynamic weights.
            nc.gpsimd.collective_compute(
                "AllToAll",
                mybir.AluOpType.bypass,
                replica_groups=[list(range(N))],
                ins=[agin[:].opt()],
                outs=[agout[:].opt()],
            )

            # ------------ stage B + stage C, software-pipelined:
            # B(0), B(1), C(0), B(2), C(1), B(3), C(2), C(3)
            S = [sb.tile([128, 9 * 128], F16, tag=f"S{ch}", name=f"S{ch}")
                 for ch in range(4)]
            PBf = [None] * 4

            def stage_b(ch):
                # gathers from agout (fp16): D on SP, PK on scalar, PB gpsimd
                D = sb.tile([128, 40], F16, tag=f"D{ch}", name=f"D{ch}")
                PKr = sb.tile([128, 8], F16, tag=f"PKr{ch}", name=f"PKr{ch}")
                PBr = sb.tile([128, 8], F16, tag=f"PBr{ch}", name=f"PBr{ch}")
                dpt = _pt(D)
                for hh in range(2):
                    base = (2 * ch + hh) * BLK
                    nc.sync.dma_start(
                        out=_ap(D, 64 * hh * dpt, [[dpt, 64], [4, 9], [1, 4]]),
                        in_=_ap(agout[:], base, [[4, 64], [OSL, 9], [1, 4]]),
                    )
                    nc.scalar.dma_start(
                        out=_ap(PKr, 64 * hh * _pt(PKr),
                                [[_pt(PKr), 64], [1, 4]]),
                        in_=_ap(agout[:], base + BPK, [[4, 64], [1, 4]]),
                    )
                    nc.gpsimd.dma_start(
                        out=_ap(PBr, 64 * hh * _pt(PBr),
                                [[_pt(PBr), 64], [1, 1]]),
                        in_=_ap(agout[:], base + BPB, [[1, 64], [1, 1]]),
                    )
                # receiver-side static biases
                nc.vector.tensor_tensor(
                    D[:, 0:36], D[:, 0:36],
                    wC(C2_BIASD + ch * 36, 128, 36),
                    op=mybir.AluOpType.add,
                )
                PKb = sb.tile([128, 4], F32, tag=f"PKb{ch}", name=f"PKb{ch}")
                nc.vector.tensor_tensor(
                    PKb[:, :], PKr[:, 0:4],
                    wC(C2_BIASPK + ch * 4, 128, 4),
                    op=mybir.AluOpType.add,
                )
                pbf = sb.tile([128, 1], F32, tag=f"PBf{ch}", name=f"PBf{ch}")
                nc.vector.tensor_tensor(
                    pbf[:, :], PBr[:, 0:1],
                    wC(C2_BIASPB + ch, 128, 1),
                    op=mybir.AluOpType.add,
                )
                PBf[ch] = pbf
                # W_eff = sum_m PK[:,m] * (perm_m @ D)
                dp = psb.tile([128, 144], F32, tag="sB", name="dp")
                for m2 in range(4):
                    nc.tensor.matmul(
                        dp[:, m2 * 36:(m2 + 1) * 36],
                        wC(C2_PERM + m2 * 128, 128, 128),
                        D[:, 0:36],
                        start=True,
                        stop=True,
                    )
                wef = sb.tile([128, 36], F16, tag=f"wef{ch}", name=f"wef{ch}")
                tmp = sb.tile([128, 36], F16, tag=f"wtm{ch}", name=f"wtm{ch}")
                nc.vector.tensor_scalar_mul(wef[:, :], dp[:, 0:36], PKb[:, 0:1])
                for m2 in range(1, 4):
                    nc.vector.tensor_scalar_mul(
                        tmp[:, :], dp[:, m2 * 36:(m2 + 1) * 36], PKb[:, m2:m2 + 1]
                    )
                    nc.vector.tensor_add(wef[:, :], wef[:, :], tmp[:, :])
                # expand W_eff -> block-diag S via transpose + select-matmuls
                tpp = psb.tile([36, 128], F16, tag="sB", name="tpp")
                nc.tensor.matmul(
                    tpp[:, :], wef[:, :], wC(C2_IDENT, 128, 128),
                    is_transpose=True, start=True, stop=True,
                )
                wefT = sb.tile([36, 128], F16, tag=f"wefT{ch}", name=f"wefT{ch}")
                nc.vector.tensor_copy(wefT[:, :], tpp[:, :])
                for grp in range(3):
                    sps = psb.tile([128, 3 * 128], F32, tag="sB", name="sps")
                    for tt in range(3):
                        t = grp * 3 + tt
                        nc.tensor.matmul(
                            sps[:, tt * 128:(tt + 1) * 128],
                            wC(C2_SEL + t * 128, 36, 128),
                            wefT[:, :],
                            start=True, stop=True,
                        )
                    nc.vector.tensor_tensor(
                        S[ch][:, grp * 384:(grp + 1) * 384], sps[:, :],
                        wC(C2_MASK, 128, 384),
                        op=mybir.AluOpType.mult,
                    )

            def stage_c(ch):
                osb = sbo.tile([128, HW * HW], F16, tag="osb", name="osb")
                for wave in range(4):
                    pcs = [psc.tile([128, 512], F32, tag="pc", name="pc")
                           for _ in range(2)]
                    for tap in range(9):
                        di, dj = tap // 3, tap % 3
                        lhs = S[ch][:, tap * 128:(tap + 1) * 128]
                        for kk, pct in enumerate(pcs):
                            r0 = (wave * 2 + kk) * 8
                            rhs = _ap(xpx, ch * XPW + (r0 + di) * HWP + dj,
                                      [[4 * XPW, 128], [HWP, 8], [1, HW]])
                            nc.tensor.matmul(
                                pct[:, :],
                                lhs,
                                rhs,
                                start=(tap == 0),
                                stop=(tap == 8),
                            )
                    for kk, pct in enumerate(pcs):
                        s8 = wave * 2 + kk
                        nc.scalar.activation(
                            osb[:, s8 * 512:(s8 + 1) * 512], pct[:, :], IDENT,
                            bias=PBf[ch][:, 0:1],
                        )
                    if wave % 2 == 1:
                        h0 = (wave - 1) * 1024
                        nc.sync.dma_start(
                            out=out[ch * 128:(ch + 1) * 128, h0:h0 + 2048],
                            in_=osb[:, h0:h0 + 2048],
                        )

            stage_b(0)
            stage_b(1)
            stage_c(0)
            stage_b(2)
            stage_c(1)
            stage_b(3)
            stage_c(2)
            stage_c(3)

    _split_excess_waits(nc)
    return nc


_NC_CACHE = {}


def _get_nc():
    if "nc" not in _NC_CACHE:
        _NC_CACHE["nc"] = build_nc()
    return _NC_CACHE["nc"]


def _pack128(arr):
    """[512, X] -> [128, 4*X] with free idx = blk*X + x."""
    xw = arr.shape[1]
    return np.ascontiguousarray(
        arr.reshape(4, 128, xw).transpose(1, 0, 2).reshape(128, 4 * xw))


def make_in_maps(inputs):
    """Host-side shard/layout prep (cast + layout only)."""
    f16 = np.float16
    style = np.asarray(inputs["style_encoding"], np.float32)
    pred = np.asarray(inputs["predicted"], np.float32)
    w1 = np.asarray(inputs["dw1_w"], np.float32).reshape(512, 512)
    w2 = np.asarray(inputs["dw2_w"], np.float32).reshape(2048, 512, 2, 2)
    pk1 = np.asarray(inputs["pk1_w"], np.float32).reshape(512, 512)
    pk2 = np.asarray(inputs["pk2_w"], np.float32).reshape(2048, 512)
    pb1 = np.asarray(inputs["pb1_w"], np.float32).reshape(512, 512)
    pb2 = np.asarray(inputs["pb2_w"], np.float32).reshape(512, 512)
    b1 = np.asarray(inputs["dw1_b"], np.float32)
    b2 = np.asarray(inputs["dw2_b"], np.float32)
    bk1 = np.asarray(inputs["pk1_b"], np.float32)
    bk2 = np.asarray(inputs["pk2_b"], np.float32)
    bb1 = np.asarray(inputs["pb1_b"], np.float32)
    bb2 = np.asarray(inputs["pb2_b"], np.float32)

    # ---- wallA (shared): st4 | w1 | b1t
    st4 = _pack128(style.transpose(1, 0, 2, 3).reshape(512, N * NPOS))
    w1p = _pack128(np.ascontiguousarray(w1.T))
    b1t = b1.reshape(4, 128).T
    wallA = np.concatenate([st4, w1p, b1t], axis=1).astype(f16)
    assert wallA.shape[1] == WA, wallA.shape

    # ---- wallB (per-core only in w2m/pwm slices)
    # fold the 1/16 spatial mean into the first pooled layer's weights
    pw1p = _pack128(np.ascontiguousarray(
        np.concatenate([pk1.T, pb1.T], axis=1) * (1.0 / NPOS)))
    pwb1 = np.concatenate(
        [bk1.reshape(4, 128).T, bb1.reshape(4, 128).T], axis=1)
    permm = np.zeros((4, 128, 128), np.float32)
    for m2 in range(4):
        for p in range(128):
            permm[m2, 4 * (p // 4) + m2, p] = 1.0
    permm = permm.transpose(1, 0, 2).reshape(128, 512)
    identm = np.eye(128, dtype=np.float32)
    # selm rows k2 = t2*4 + i2 (t-major, matching D's free layout), padded
    selm = np.zeros((36, 9, 128), np.float32)
    for t in range(9):
        for p in range(128):
            selm[t * 4 + (p % 4), t, p] = 1.0
    selm = np.concatenate(
        [selm.reshape(36, 9 * 128), np.zeros((92, 9 * 128), np.float32)], 0)
    maskm = np.zeros((128, 128), np.float32)
    for p in range(128):
        maskm[p, 4 * (p // 4):4 * (p // 4) + 4] = 1.0
    maskm = np.tile(maskm, (1, 3))
    biasD = np.broadcast_to(
        b2.reshape(512, 4)[:, None, :], (512, 9, 4)).reshape(512, 36)
    biasD = _pack128(biasD)
    biasPK = _pack128(bk2.reshape(512, 4))
    biasPB = bb2.reshape(4, 128).T
    wallB2_ = np.ascontiguousarray(np.concatenate(
        [permm, identm, selm, maskm, biasD, biasPK, biasPB],
        axis=1)).astype(f16)
    assert wallB2_.shape[1] == WB2, wallB2_.shape

    # padded input, per core
    xpad_all = np.pad(pred, ((0, 0), (0, 0), (1, 1), (1, 1)), mode="reflect")
    xpad_all = xpad_all.reshape(N, 512, HWP * HWP).astype(f16)

    in_maps = []
    for c in range(N):
        xz = np.zeros((512, XPW), f16)
        xz[:, :HWP * HWP] = xpad_all[c]
        xz = np.ascontiguousarray(
            xz.reshape(4, 128, XPW).transpose(1, 0, 2).reshape(128, 4 * XPW))
        # dw2 slice, weights-as-moving layout [128, (ib, dydx, o)]
        w2s = w2[c * OSL:(c + 1) * OSL]          # [256, 512, 2, 2]
        w2m_ = w2s.transpose(1, 2, 3, 0)         # [512, 2, 2, 256]
        w2m_ = (w2m_.reshape(4, 128, 2, 2, OSL)
                .transpose(1, 0, 2, 3, 4)
                .reshape(128, 16 * OSL))
        # pooled layer-2 moving [128, (it, [pk 256 | pb 64])]
        pk2s = pk2[c * PKSL:(c + 1) * PKSL].T    # [512, 256]
        pb2s = pb2[c * PBSL:(c + 1) * PBSL].T    # [512, 64]
        pwm_ = np.concatenate([pk2s, pb2s], axis=1)   # [512, 320]
        pwm_ = (pwm_.reshape(4, 128, 320)
                .transpose(1, 0, 2)
                .reshape(128, 4 * 320))
        wallB1_ = np.concatenate(
            [pw1p, pwb1, w2m_, pwm_], axis=1).astype(f16)
        assert wallB1_.shape[1] == WB1, wallB1_.shape
        in_maps.append({
            "wallA": wallA,
            "wallB1": np.ascontiguousarray(wallB1_),
            "wallB2": wallB2_,
            "xpad": xz,
        })
    return in_maps


def kernel(**inputs):
    install_profile_shim()
    from concourse.bass_utils import run_bass_kernel_spmd

    nc = _get_nc()
    in_maps = make_in_maps(inputs)
    res = run_bass_kernel_spmd(nc, in_maps, core_ids=list(range(N)))
    outs = [np.asarray(res.results[c]["out"]).reshape(COUT, HW, HW)
            for c in range(N)]
    return np.stack(outs, axis=0).astype(np.float32)


# revision 14
# speedup vs baseline: 2.1812x; 1.1649x over previous
"""AdaConv Trainium2 kernel — 8-core SPMD, data-parallel over batch.

v5: fully local per-core computation — NO collective. Each core receives
only ITS sample's style/image and the FULL (replicated) prediction-net
weights, and computes its own dynamic conv weights locally. The AllToAll
of v3/v4 cost ~40us in CC mesh setup; replicating w2/pk2/pb2 (+9.5MiB
fp16 DMA, hidden under compute) and spending ~19us extra PE time on the
unsliced layer-2 matmuls removes it entirely and kills all cross-core
sync variance.

Structure:
  * All dynamic tensors fp16 (full PE rate), fp16 output upcast on host.
  * Weights/constants host-packed into few [128, W] walls; w2 loaded in
    4 chunk-quarters so stage C chunk 0 starts while later quarters
    stream. dma_start costs ~650ns serial issue time per engine, so
    issues are few and spread over SP/scalar/gpsimd queues.
  * Stage A (own sample): layer-1 h via 16 N=16 matmuls, lrelu fused in
    windowed psum drains (im2col layout for dw2); dw2 weights-as-moving
    (16 N=512 MMs per chunk-quarter, psum [9,512]); pooled path via
    M=1 stationaries. Static layer-2 biases folded receiver-side.
  * Dynamic weights roundtrip through tiny DRAM scratch so the
    (channel, tap) deinterleave is a single 3-D DMA gather per chunk.
  * Stage B: block-diag fp16 stationaries S[ch] via perm-matmuls +
    per-partition scales + select-matmuls + mask.
  * Stage C: grouped 3x3 conv, 9 psum-accumulated fp16 matmuls per
    2-sub wave, 4 rotating psum banks; per-half-chunk output stores.
  * Software pipeline: dw2(q)/B(q) run one chunk ahead of C(q).
"""
import sys
import types

sys.path.insert(0, "/opt/trn_rl_repo")

import numpy as np

import concourse.bass as bass
import concourse.mybir as mybir

N = 8          # batch == cores
CIN = 512
COUT = 512
HW = 64        # spatial
HWP = 66       # padded
XPW = 4384     # per-chunk padded width (66*66=4356 used)
NPOS = 16      # style spatial 4x4

# wallA layout (cols): st-own | w1 | b1t
CA_ST = 0
CA_W1 = CA_ST + NPOS * 4
CA_B1 = CA_W1 + 2048
WA = CA_B1 + 4            # 2116
# wallP1: pooled layer-1 weights
CP_PW1 = 0
CP_PWB1 = CP_PW1 + 4096
WP1 = CP_PWB1 + 8         # 4104
# wallP2: pooled layer-2 moving (pk2.T | pb2.T packed)
WP2 = 4 * 2560            # 10240
# wallP3: stage-B constants
C2_PERM = 0
C2_IDENT = C2_PERM + 512
C2_SEL = C2_IDENT + 128
C2_MASK = C2_SEL + 1152
C2_BIASD = C2_MASK + 384
C2_BIASPK = C2_BIASD + 144
C2_BIASPB = C2_BIASPK + 16
WP3 = C2_BIASPB + 4       # 2340

F32 = mybir.dt.float32
F16 = mybir.dt.float16


# ---------------------------------------------------------------- tile patch
def _install_tile_patch():
    """walrus here rejects Drain instructions with >1 sync-wait; spread the
    Tile tail-drain waits over individual SP nops."""
    import concourse.tile as tile_mod
    from concourse.vector_clock import ScopedClock

    def _patched(self, tick_clock, wait_clock):
        nc = self.nc
        drain_inst = nc.sync.drain()
        wait_clock.add_sem_waits(
            drain_inst.ins, ScopedClock({None: tick_clock.global_clock})
        )
        waits = list(drain_inst.ins.sync_info.on_wait or [])
        if len(waits) > 1:
            drain_inst.ins.sync_info.on_wait = waits[:1]
            for w in waits[1:]:
                nop = nc.sync.nop(nofuse=True, hint="tail_wait_split")
                if nop.ins.sync_info is None:
                    nop.ins.sync_info = mybir.SyncInfo(on_wait=[w], on_update=[])
                else:
                    nop.ins.sync_info.on_wait = [w]
        nc.all_engine_barrier()
        assert self.sems is not None
        popped = nc._tile_sem_poison_stack.pop()
        assert popped is self._sem_poison
        nc.clear_and_free_semaphores(list(self.sems.allocated().values()))
        nc.all_engine_barrier()

    tile_mod.TileContext._drain_and_barrier = _patched


_install_tile_patch()
from concourse.tile import TileContext  # noqa: E402


def install_profile_shim():
    """antenv.axon_hooks is missing from this image; recreate it so
    run_bass_kernel_spmd(trace=True) can capture NTFF profiles."""
    if "antenv.axon_hooks" in sys.modules:
        return
    import antenv

    mod = types.ModuleType("antenv.axon_hooks")
    mod._hook = None
    mod.set_axon_ntff_profile_hook = lambda h: setattr(mod, "_hook", h)
    mod.get_axon_ntff_profile_hook = lambda: mod._hook
    sys.modules["antenv.axon_hooks"] = mod
    antenv.axon_hooks = mod
    try:
        if "/root/.axon_site" not in sys.path:
            sys.path.insert(0, "/root/.axon_site")
        from trn_agent_boot.trn_boot import _ntff_profile_via_ctypes

        hook = _ntff_profile_via_ctypes("/opt/axon/libaxon_pjrt.so")
        mod.set_axon_ntff_profile_hook(hook)
    except Exception:
        pass


def _ap(t_ap, offset, dims):
    """Custom flat AP over a tile's underlying tensor."""
    return bass.AP(t_ap.tensor, offset, [list(d) for d in dims])


def _pt(t):
    """Physical partition pitch (elements) of a tile."""
    return t[:, :].ap[0][0]


def _split_excess_waits(nc, max_waits=1):
    """This walrus build rejects instructions carrying more than ~1 sync-wait.
    Move excess waits onto same-engine NoOps inserted just before."""
    n_split = 0
    for f in nc.m.functions:
        for bb in f.blocks:
            newlist = []
            for inst in bb.instructions:
                si = getattr(inst, "sync_info", None)
                if si is not None and si.on_wait and len(si.on_wait) > max_waits:
                    waits = list(si.on_wait)
                    for k, w in enumerate(waits[max_waits:]):
                        nop = mybir.InstNoOp(
                            name=f"{inst.name}_ws{k}",
                            engine=inst.engine,
                            bass_nofuse=True,
                            sync_info=mybir.SyncInfo(on_wait=[w], on_update=[]),
                        )
                        newlist.append(nop)
                        n_split += 1
                    si.on_wait = waits[:max_waits]
                newlist.append(inst)
            try:
                bb.instructions[:] = newlist
            except TypeError:
                bb.set_instructions(newlist)
    return n_split


LRELU = mybir.ActivationFunctionType.Lrelu
IDENT = mybir.ActivationFunctionType.Identity
COPY = mybir.ActivationFunctionType.Copy


def build_nc():
    nc = bass.Bass(target_bir_lowering=False)

    wallA = nc.declare_dram_parameter("wallA", [128, WA], F16, isOutput=False)
    wallP1 = nc.declare_dram_parameter("wallP1", [128, WP1], F16, isOutput=False)
    wallP2 = nc.declare_dram_parameter("wallP2", [128, WP2], F16, isOutput=False)
    wallP3 = nc.declare_dram_parameter("wallP3", [128, WP3], F16, isOutput=False)
    w2q = [nc.declare_dram_parameter(f"w2q{q}", [128, 16 * 512], F16,
                                     isOutput=False) for q in range(4)]
    xpad = nc.declare_dram_parameter("xpad", [128, 4 * XPW], F16, isOutput=False)
    out = nc.declare_dram_parameter("out", [COUT, HW * HW], F16, isOutput=True)

    with TileContext(nc) as tc:
        with (
            tc.tile_pool(name="sb", bufs=1) as sb,
            tc.tile_pool(name="sbx", bufs=1) as sbx,
            tc.tile_pool(name="sbo", bufs=2) as sbo,
            tc.tile_pool(name="psb", bufs=2, space="PSUM") as psb,
            tc.tile_pool(name="psc", bufs=4, space="PSUM") as psc,
            tc.tile_pool(name="dram", bufs=1, space="DRAM") as dram,
        ):
            wa = sb.tile([128, WA], F16, tag="wa", name="wa")
            nc.sync.dma_start(out=wa[:, :], in_=wallA[:, :])
            wp = sb.tile([128, WP1], F16, tag="wp", name="wp")
            nc.sync.dma_start(out=wp[:, :], in_=wallP1[:, :])
            wm = sb.tile([128, WP2], F16, tag="wm", name="wm")
            nc.sync.dma_start(out=wm[:, :], in_=wallP2[:, :])
            w2sb = [sb.tile([128, 16 * 512], F16, tag=f"w2sb{q}",
                            name=f"w2sb{q}") for q in range(4)]
            nc.sync.dma_start(out=w2sb[0][:, :], in_=w2q[0][:, :])
            wc = sb.tile([128, WP3], F16, tag="wc", name="wc")
            nc.sync.dma_start(out=wc[:, :], in_=wallP3[:, :])
            xpx = sbx.tile([128, 4 * XPW], F16, tag="xpx", name="xpx")
            nc.sync.dma_start(out=xpx[:, :], in_=xpad[:, :])
            for q in range(1, 4):
                nc.sync.dma_start(out=w2sb[q][:, :], in_=w2q[q][:, :])
            wap, wpp, wmp, wcp = _pt(wa), _pt(wp), _pt(wm), _pt(wc)

            def wA(col, np_, nf):
                return _ap(wa, col, [[wap, np_], [1, nf]])

            def wP(col, np_, nf):
                return _ap(wp, col, [[wpp, np_], [1, nf]])

            def wM(col, np_, nf):
                return _ap(wm, col, [[wmp, np_], [1, nf]])

            def wC(col, np_, nf):
                return _ap(wc, col, [[wcp, np_], [1, nf]])

            # ------------ stage A: h = lrelu(W1 s + b1) for OWN sample,
            # drained into im2col h2[ot][:, dydx*9:+9] = (ty, tx) windows
            h2 = [sb.tile([128, 36], F16, tag=f"h2{ot}", name=f"h2{ot}")
                  for ot in range(4)]
            for ot in range(4):
                pa = psb.tile([128, NPOS], F32, tag="sA", name="pa")
                for it in range(4):
                    nc.tensor.matmul(
                        pa[:, :],
                        wA(CA_W1 + it * CIN + ot * 128, 128, 128),
                        wA(CA_ST + it * NPOS, 128, NPOS),
                        start=(it == 0),
                        stop=(it == 3),
                    )
                pap = _pt(pa)
                for dy in range(2):
                    for dx in range(2):
                        nc.scalar.activation(
                            h2[ot][:, (dy * 2 + dx) * 9:(dy * 2 + dx + 1) * 9],
                            _ap(pa, dy * 4 + dx, [[pap, 128], [4, 3], [1, 3]]),
                            LRELU,
                            bias=wA(CA_B1 + ot, 128, 1), alpha=0.01,
                        )

            # ------------ stage A: pooled path, own sample (width-1)
            sp = [sb.tile([128, 1], F16, tag=f"sp{i}", name=f"sp{i}")
                  for i in range(4)]
            with nc.allow_low_precision("16-term style pool in fp16"):
                for i in range(4):
                    nc.vector.tensor_reduce(
                        sp[i][:, :],
                        _ap(wa, CA_ST + i * NPOS, [[wap, 128], [1, NPOS]]),
                        axis=mybir.AxisListType.X,
                        op=mybir.AluOpType.add,
                    )
            ac = []
            for po in range(8):
                pp = psb.tile([128, 1], F32, tag="sA", name="pp")
                for it in range(4):
                    nc.tensor.matmul(
                        pp[:, :],
                        wP(CP_PW1 + it * 2 * CIN + po * 128, 128, 128),
                        sp[it][:, :],
                        start=(it == 0),
                        stop=(it == 3),
                    )
                a = sb.tile([128, 1], F16, tag=f"ac{po}", name=f"ac{po}")
                nc.scalar.activation(
                    a[:, :], pp[:, :], LRELU,
                    bias=wP(CP_PWB1 + po, 128, 1), alpha=0.01,
                )
                ac.append(a)
            # pk2|pb2 own-sample: psum [1, 512] x5 segs (4 pk + 1 pb)
            pks = dram.tile([2560], F16)
            pkbo = sb.tile([1, 2560], F16, tag="pkbo", name="pkbo")
            for seg in range(5):
                pko = psb.tile([1, 512], F32, tag="sA", name="pko")
                for it in range(4):
                    nc.tensor.matmul(
                        pko[:, :],
                        ac[it if seg < 4 else 4 + it][:, :],
                        wM(it * 2560 + seg * 512, 128, 512),
                        start=(it == 0),
                        stop=(it == 3),
                    )
                nc.scalar.activation(
                    pkbo[:, seg * 512:(seg + 1) * 512], pko[:, :], COPY)
            nc.scalar.dma_start(
                out=_ap(pks[:], 0, [[1, 2560]]), in_=pkbo[:, :])

            # ------------ dw2 per chunk-quarter (weights-as-moving),
            # psum [9=(ty,tx), 512=k-slice] -> DRAM scratch
            dws = [dram.tile([9 * 512], F16, name=f"dws{q}")
                   for q in range(4)]

            def dw2_q(q):
                pd = psb.tile([9, 512], F32, tag="sA", name="pd")
                k = 0
                for ib in range(4):
                    for dydx in range(4):
                        nc.tensor.matmul(
                            pd[:, :],
                            h2[ib][:, dydx * 9:(dydx + 1) * 9],
                            _ap(w2sb[q], (ib * 4 + dydx) * 512,
                                [[16 * 512, 128], [1, 512]]),
                            start=(k == 0),
                            stop=(k == 15),
                        )
                        k += 1
                dwo = sb.tile([9, 512], F16, tag="dwo", name="dwo")
                nc.scalar.activation(dwo[:, :], pd[:, :], COPY)
                nc.scalar.dma_start(
                    out=_ap(dws[q][:], 0, [[512, 9], [1, 512]]),
                    in_=dwo[:, :],
                )

            # ------------ stage B + stage C
            S = [sb.tile([128, 9 * 128], F16, tag=f"S{ch}", name=f"S{ch}")
                 for ch in range(4)]
            PBf = [None] * 4

            def stage_b(ch):
                # gathers from local scratch: D on SP, PK scalar, PB gpsimd
                D = sb.tile([128, 40], F16, tag=f"D{ch}", name=f"D{ch}")
                PKr = sb.tile([128, 8], F16, tag=f"PKr{ch}", name=f"PKr{ch}")
                PBr = sb.tile([128, 8], F16, tag=f"PBr{ch}", name=f"PBr{ch}")
                dpt = _pt(D)
                nc.sync.dma_start(
                    out=_ap(D, 0, [[dpt, 128], [4, 9], [1, 4]]),
                    in_=_ap(dws[ch][:], 0, [[4, 128], [512, 9], [1, 4]]),
                )
                nc.scalar.dma_start(
                    out=_ap(PKr, 0, [[_pt(PKr), 128], [1, 4]]),
                    in_=_ap(pks[:], 512 * ch, [[4, 128], [1, 4]]),
                )
                nc.gpsimd.dma_start(
                    out=_ap(PBr, 0, [[_pt(PBr), 128], [1, 1]]),
                    in_=_ap(pks[:], 2048 + 128 * ch, [[1, 128], [1, 1]]),
                )
                # receiver-side static biases
                nc.vector.tensor_tensor(
                    D[:, 0:36], D[:, 0:36],
                    wC(C2_BIASD + ch * 36, 128, 36),
                    op=mybir.AluOpType.add,
                )
                PKb = sb.tile([128, 4], F32, tag=f"PKb{ch}", name=f"PKb{ch}")
                nc.vector.tensor_tensor(
                    PKb[:, :], PKr[:, 0:4],
                    wC(C2_BIASPK + ch * 4, 128, 4),
                    op=mybir.AluOpType.add,
                )
                pbf = sb.tile([128, 1], F32, tag=f"PBf{ch}", name=f"PBf{ch}")
                nc.vector.tensor_tensor(
                    pbf[:, :], PBr[:, 0:1],
                    wC(C2_BIASPB + ch, 128, 1),
                    op=mybir.AluOpType.add,
                )
                PBf[ch] = pbf
                # W_eff = sum_m PK[:,m] * (perm_m @ D)
                dp = psb.tile([128, 144], F32, tag="sB", name="dp")
                for m2 in range(4):
                    nc.tensor.matmul(
                        dp[:, m2 * 36:(m2 + 1) * 36],
                        wC(C2_PERM + m2 * 128, 128, 128),
                        D[:, 0:36],
                        start=True,
                        stop=True,
                    )
                wef = sb.tile([128, 36], F16, tag=f"wef{ch}", name=f"wef{ch}")
                tmp = sb.tile([128, 36], F16, tag=f"wtm{ch}", name=f"wtm{ch}")
                nc.vector.tensor_scalar_mul(wef[:, :], dp[:, 0:36], PKb[:, 0:1])
                for m2 in range(1, 4):
                    nc.vector.tensor_scalar_mul(
                        tmp[:, :], dp[:, m2 * 36:(m2 + 1) * 36], PKb[:, m2:m2 + 1]
                    )
                    nc.vector.tensor_add(wef[:, :], wef[:, :], tmp[:, :])
                # expand W_eff -> block-diag S via transpose + select-matmuls
                tpp = psb.tile([36, 128], F16, tag="sB", name="tpp")
                nc.tensor.matmul(
                    tpp[:, :], wef[:, :], wC(C2_IDENT, 128, 128),
                    is_transpose=True, start=True, stop=True,
                )
                wefT = sb.tile([36, 128], F16, tag=f"wefT{ch}", name=f"wefT{ch}")
                nc.vector.tensor_copy(wefT[:, :], tpp[:, :])
                for grp in range(3):
                    sps = psb.tile([128, 3 * 128], F32, tag="sB", name="sps")
                    for tt in range(3):
                        t = grp * 3 + tt
                        nc.tensor.matmul(
                            sps[:, tt * 128:(tt + 1) * 128],
                            wC(C2_SEL + t * 128, 36, 128),
                            wefT[:, :],
                            start=True, stop=True,
                        )
                    nc.vector.tensor_tensor(
                        S[ch][:, grp * 384:(grp + 1) * 384], sps[:, :],
                        wC(C2_MASK, 128, 384),
                        op=mybir.AluOpType.mult,
                    )

            def stage_c(ch):
                osb = sbo.tile([128, HW * HW], F16, tag="osb", name="osb")
                for wave in range(4):
                    pcs = [psc.tile([128, 512], F32, tag="pc", name="pc")
                           for _ in range(2)]
                    for tap in range(9):
                        di, dj = tap // 3, tap % 3
                        lhs = S[ch][:, tap * 128:(tap + 1) * 128]
                        for kk, pct in enumerate(pcs):
                            r0 = (wave * 2 + kk) * 8
                            rhs = _ap(xpx, ch * XPW + (r0 + di) * HWP + dj,
                                      [[4 * XPW, 128], [HWP, 8], [1, HW]])
                            nc.tensor.matmul(
                                pct[:, :],
                                lhs,
                                rhs,
                                start=(tap == 0),
                                stop=(tap == 8),
                            )
                    for kk, pct in enumerate(pcs):
                        s8 = wave * 2 + kk
                        nc.scalar.activation(
                            osb[:, s8 * 512:(s8 + 1) * 512], pct[:, :], IDENT,
                            bias=PBf[ch][:, 0:1],
                        )
                    if wave % 2 == 1:
                        h0 = (wave - 1) * 1024
                        nc.sync.dma_start(
                            out=out[ch * 128:(ch + 1) * 128, h0:h0 + 2048],
                            in_=osb[:, h0:h0 + 2048],
                        )

            dw2_q(0)
            stage_b(0)
            dw2_q(1)
            stage_b(1)
            stage_c(0)
            dw2_q(2)
            stage_b(2)
            stage_c(1)
            dw2_q(3)
            stage_b(3)
            stage_c(2)
            stage_c(3)

    _split_excess_waits(nc)
    return nc


_NC_CACHE = {}


def _get_nc():
    if "nc" not in _NC_CACHE:
        _NC_CACHE["nc"] = build_nc()
    return _NC_CACHE["nc"]


def _pack128(arr):
    """[512, X] -> [128, 4*X] with free idx = blk*X + x."""
    xw = arr.shape[1]
    return np.ascontiguousarray(
        arr.reshape(4, 128, xw).transpose(1, 0, 2).reshape(128, 4 * xw))


def make_in_maps(inputs):
    """Host-side shard/layout prep (cast + layout only)."""
    f16 = np.float16
    style = np.asarray(inputs["style_encoding"], np.float32)
    pred = np.asarray(inputs["predicted"], np.float32)
    w1 = np.asarray(inputs["dw1_w"], np.float32).reshape(512, 512)
    w2 = np.asarray(inputs["dw2_w"], np.float32).reshape(2048, 512, 2, 2)
    pk1 = np.asarray(inputs["pk1_w"], np.float32).reshape(512, 512)
    pk2 = np.asarray(inputs["pk2_w"], np.float32).reshape(2048, 512)
    pb1 = np.asarray(inputs["pb1_w"], np.float32).reshape(512, 512)
    pb2 = np.asarray(inputs["pb2_w"], np.float32).reshape(512, 512)
    b1 = np.asarray(inputs["dw1_b"], np.float32)
    b2 = np.asarray(inputs["dw2_b"], np.float32)
    bk1 = np.asarray(inputs["pk1_b"], np.float32)
    bk2 = np.asarray(inputs["pk2_b"], np.float32)
    bb1 = np.asarray(inputs["pb1_b"], np.float32)
    bb2 = np.asarray(inputs["pb2_b"], np.float32)

    # ---- shared walls
    w1p = _pack128(np.ascontiguousarray(w1.T))
    b1t = b1.reshape(4, 128).T
    # fold the 1/16 spatial mean into the first pooled layer's weights
    pw1p = _pack128(np.ascontiguousarray(
        np.concatenate([pk1.T, pb1.T], axis=1) * (1.0 / NPOS)))
    pwb1 = np.concatenate(
        [bk1.reshape(4, 128).T, bb1.reshape(4, 128).T], axis=1)
    wallP1 = np.ascontiguousarray(
        np.concatenate([pw1p, pwb1], axis=1)).astype(f16)
    assert wallP1.shape[1] == WP1
    wallP2 = _pack128(np.ascontiguousarray(
        np.concatenate([pk2.T, pb2.T], axis=1))).astype(f16)
    assert wallP2.shape[1] == WP2

    permm = np.zeros((4, 128, 128), np.float32)
    for m2 in range(4):
        for p in range(128):
            permm[m2, 4 * (p // 4) + m2, p] = 1.0
    permm = permm.transpose(1, 0, 2).reshape(128, 512)
    identm = np.eye(128, dtype=np.float32)
    # selm rows k2 = t2*4 + i2 (t-major, matching D's free layout), padded
    selm = np.zeros((36, 9, 128), np.float32)
    for t in range(9):
        for p in range(128):
            selm[t * 4 + (p % 4), t, p] = 1.0
    selm = np.concatenate(
        [selm.reshape(36, 9 * 128), np.zeros((92, 9 * 128), np.float32)], 0)
    maskm = np.zeros((128, 128), np.float32)
    for p in range(128):
        maskm[p, 4 * (p // 4):4 * (p // 4) + 4] = 1.0
    maskm = np.tile(maskm, (1, 3))
    biasD = np.broadcast_to(
        b2.reshape(512, 4)[:, None, :], (512, 9, 4)).reshape(512, 36)
    biasD = _pack128(biasD)
    biasPK = _pack128(bk2.reshape(512, 4))
    biasPB = bb2.reshape(4, 128).T
    wallP3 = np.ascontiguousarray(np.concatenate(
        [permm, identm, selm, maskm, biasD, biasPK, biasPB],
        axis=1)).astype(f16)
    assert wallP3.shape[1] == WP3

    # w2 quarters (shared): [128, (ib, dydx, k-slice 512)]
    w2qs = []
    for q in range(4):
        w2s = w2[q * 512:(q + 1) * 512]          # [512, 512, 2, 2]
        w2m_ = w2s.transpose(1, 2, 3, 0)         # [512i, 2, 2, 512k]
        w2m_ = (w2m_.reshape(4, 128, 2, 2, 512)
                .transpose(1, 0, 2, 3, 4)
                .reshape(128, 16 * 512))
        w2qs.append(np.ascontiguousarray(w2m_).astype(f16))

    # padded input, per core
    xpad_all = np.pad(pred, ((0, 0), (0, 0), (1, 1), (1, 1)), mode="reflect")
    xpad_all = xpad_all.reshape(N, 512, HWP * HWP).astype(f16)
    st_all = style.transpose(0, 2, 3, 1).reshape(N, NPOS, 512)

    in_maps = []
    for c in range(N):
        xz = np.zeros((512, XPW), f16)
        xz[:, :HWP * HWP] = xpad_all[c]
        xz = np.ascontiguousarray(
            xz.reshape(4, 128, XPW).transpose(1, 0, 2).reshape(128, 4 * XPW))
        # own-sample style [512, 16] -> [128, 4*16]
        st_own = _pack128(np.ascontiguousarray(st_all[c].T))
        wallA = np.concatenate([st_own, w1p, b1t], axis=1).astype(f16)
        assert wallA.shape[1] == WA
        m = {
            "wallA": np.ascontiguousarray(wallA),
            "wallP1": wallP1,
            "wallP2": wallP2,
            "wallP3": wallP3,
            "xpad": xz,
        }
        for q in range(4):
            m[f"w2q{q}"] = w2qs[q]
        in_maps.append(m)
    return in_maps


def kernel(**inputs):
    install_profile_shim()
    from concourse.bass_utils import run_bass_kernel_spmd

    nc = _get_nc()
    in_maps = make_in_maps(inputs)
    res = run_bass_kernel_spmd(nc, in_maps, core_ids=list(range(N)))
    outs = [np.asarray(res.results[c]["out"]).reshape(COUT, HW, HW)
            for c in range(N)]
    return np.stack(outs, axis=0).astype(np.float32)
